# revision 36
# baseline (speedup 1.0000x reference)
"""2-layer GCN (GCNConv+relu x2, linear head) on 8 Trainium2 NeuronCores.

Strategy (graph/data parallel, per sharding hint):
  - Nodes sharded across 8 cores by id; edges partitioned by destination.
  - Per core, destination nodes are bin-packed into B_FIX blocks of <=BLK
    dsts such that each (block, source-window) holds <= KCOL*128 edges.
    This gives an SPMD-uniform program; only tensor data varies per core.
  - Per layer: local matmul (x@W scaled by dinv) -> AllGather into a
    full node-major bf16 table in DRAM -> dma_gather one 256B element
    per edge = a PAIR of bf16 rows (slots 2w, 2w+1); parity-split
    selection matrices S_even/S_odd (is_equal on DVE, bf16) route the
    correct half; PE bf16 matmuls accumulate feature-major conv output
    in PSUM; self-loop terms enter via an identity-matmul transpose.
    Post: scale by dinv, +bias, relu, next-layer matmul (bf16).
  - The 4 source-window gathers go to 4 SWDGE queues: each queue's
    descriptor generation runs on its own GpSimd Q7 core pair, so the
    4 gathers of a batch overlap (queue 0 issued last since its pair
    is the one the engine timeline blocks on).
"""

import numpy as np

import concourse.bass as bass
import concourse.mybir as mybir
import concourse.tile as tile
from concourse import bacc
from concourse import bass_utils

import ml_dtypes

F32 = mybir.dt.float32
BF16 = mybir.dt.bfloat16
I16 = mybir.dt.int16
NP_BF16 = ml_dtypes.bfloat16


class Cfg:
    def __init__(self, n_nodes, in_feat, hidden, n_classes, n_cores, n_c,
                 blk, kcol, b_fix, nq, c_batch, self_dtype="bf16"):
        self.N = n_nodes
        self.IN_FEAT = in_feat
        self.HIDDEN = hidden
        self.N_CLASSES = n_classes
        self.NC = n_cores
        self.N_C = n_c                    # nodes per core (id // N_C)
        assert n_c * n_cores >= n_nodes
        self.BLK = blk                    # max dsts per block
        self.KCOL = kcol                  # columns per (block, stream)
        self.CAP = kcol * 128             # max edges per (block, stream)
        self.B_FIX = b_fix                # blocks per core (uniform)
        self.NQ = nq                      # source windows / gather streams
        self.SLOTS_C = b_fix * blk        # table slots per core
        assert self.SLOTS_C % 128 == 0
        self.NT = self.SLOTS_C // 128     # node tiles per core
        assert self.NT % 2 == 0
        self.TABLE_N = n_cores * self.SLOTS_C
        assert self.TABLE_N % nq == 0
        self.WIN = self.TABLE_N // nq     # table rows per source window
        assert self.WIN <= 32767          # int16 gather index range
        assert (n_cores % nq) == 0
        self.COLS_Q = b_fix * kcol        # gather columns per stream
        self.C_BATCH = c_batch            # columns per gather batch
        assert c_batch % kcol == 0 and self.COLS_Q % c_batch == 0
        self.N_BATCH = self.COLS_Q // c_batch
        self.BPB = c_batch // kcol        # blocks per batch
        assert self.BPB % 2 == 0          # block pairs never straddle batches
        self.SELF_DT = BF16 if self_dtype == "bf16" else F32
        self.NP_SELF = NP_BF16 if self_dtype == "bf16" else np.float32


CFG_FULL = Cfg(n_nodes=100000, in_feat=128, hidden=64, n_classes=16,
               n_cores=8, n_c=12544, blk=64, kcol=2, b_fix=224, nq=4,
               c_batch=28)


# ---------------------------------------------------------------------------
# Host-side preprocessing (sharding): all integer graph restructuring.
# ---------------------------------------------------------------------------

def preprocess(cfg, x, edge_index, W1, b1, W2, b2, Wl, bl):
    N, NC, N_C = cfg.N, cfg.NC, cfg.N_C
    src = np.asarray(edge_index[0]).astype(np.int64)
    dst = np.asarray(edge_index[1]).astype(np.int64)
    x = np.asarray(x, dtype=np.float32)

    deg = np.bincount(dst, minlength=N).astype(np.float32) + 1.0
    dinv = (1.0 / np.sqrt(deg)).astype(np.float32)

    # stream = stripe of the SOURCE node, chosen a priori and independent
    # of its parity class: stripe k nodes get packed into blocks
    # [56k, 56(k+1)) of their core, so table window q is the contiguous
    # rows [q*TABLE_N/NQ, ...) written by the q-th chunked AllGather.
    q_of = ((src // 2) % cfg.NQ).astype(np.int64)
    p_of = (src % 2).astype(np.int64)        # a-priori source parity class

    # per-(node, q, par) incoming edge counts
    degqp = np.bincount((dst * cfg.NQ + q_of) * 2 + p_of,
                        minlength=N * cfg.NQ * 2).reshape(N, cfg.NQ, 2)

    # --- per-core first-fit-decreasing packing of dsts into blocks ---
    # Constraints: per (block, q, par) <= 128 edges (one 128-row column);
    # node with id parity p gets a block position r with r % 2 == p, so
    # slot parity == id parity (known before packing any core).
    HBLK = cfg.BLK // 2
    B_STR = cfg.B_FIX // cfg.NQ          # blocks per stripe
    node_q = ((np.arange(NC * N_C) // 2) % cfg.NQ)
    slot_of = np.full(NC * N_C, -1, dtype=np.int64)
    node_of_slot = np.full(cfg.TABLE_N, -1, dtype=np.int64)
    for c in range(NC):
        lo, hi = c * N_C, min((c + 1) * N_C, N)
        if hi <= lo:
            continue
        for k in range(cfg.NQ):
            ids = lo + np.flatnonzero(node_q[lo:hi] == k)
            dq = degqp[ids].reshape(ids.size, cfg.NQ * 2)
            order = np.argsort(-dq.max(axis=1), kind="stable")
            accs = np.zeros((B_STR, cfg.NQ * 2), dtype=np.int64)
            cnts = np.zeros((B_STR, 2), dtype=np.int64)
            nopen = 1
            for j in order:
                v = dq[j]
                g = int(ids[j])
                pj = g % 2
                fits = (cnts[:nopen, pj] < HBLK) & \
                       np.all(accs[:nopen] + v <= 128, axis=1)
                w = np.flatnonzero(fits)
                if w.size == 0:
                    assert nopen < B_STR, \
                        f"core {c} stripe {k}: packing exceeds {B_STR} blocks"
                    b = nopen
                    nopen += 1
                else:
                    b = int(w[0])
                r = 2 * cnts[b, pj] + pj
                s = c * cfg.SLOTS_C + (k * B_STR + b) * cfg.BLK + r
                slot_of[g] = s
                node_of_slot[s] = g
                accs[b] += v
                cnts[b, pj] += 1

    slot_of = slot_of[:N]

    # --- per-core edge streams ---
    e_core = dst // N_C
    s_slot = slot_of[src]
    d_slot_l = slot_of[dst] - e_core * cfg.SLOTS_C
    e_b = d_slot_l // cfg.BLK
    e_r = d_slot_l % cfg.BLK

    P_Q = cfg.B_FIX * cfg.CAP            # positions per stream
    idx_all = np.zeros((NC, cfg.NQ, P_Q), dtype=np.int16)

    e_par = (s_slot % 2).astype(np.int64)    # == src % 2 by construction
    order2 = np.lexsort((e_par, e_b, q_of, e_core))
    es_c, eq_c, eb_c = e_core[order2], q_of[order2], e_b[order2]
    ep_c = e_par[order2]
    grp = ((es_c * cfg.NQ + eq_c) * cfg.B_FIX + eb_c) * 2 + ep_c
    _, start_idx, cnt_grp = np.unique(grp, return_index=True,
                                      return_counts=True)
    rank = np.arange(grp.size) - np.repeat(start_idx, cnt_grp)
    assert rank.max(initial=0) < 128
    # column = 2*block + parity; position = column*128 + rank
    pos = eb_c * cfg.CAP + ep_c * 128 + rank
    s_sorted = s_slot[order2]
    # table-window row of the source: window q holds, for every core c,
    # that core's stripe-q local rows at [c*3584, (c+1)*3584)
    STR_ROWS = cfg.SLOTS_C // cfg.NQ
    l_sorted = s_sorted % cfg.SLOTS_C
    assert np.all(l_sorted // STR_ROWS == eq_c)
    wrow = (s_sorted // cfg.SLOTS_C) * STR_ROWS + (l_sorted % STR_ROWS)
    # gather PAIR index (two table rows per 256B element)
    idx_val = (wrow // 2).astype(np.int16)
    idx_all[es_c, eq_c, pos] = idx_val

    # wrapped int16 layout: position i -> [i%16, i//16], replicated x8
    idx_w = idx_all.reshape(NC, cfg.NQ, -1, 16).transpose(0, 1, 3, 2)
    idx_dev = np.ascontiguousarray(np.tile(idx_w, (1, 1, 8, 1)))

    # host-built one-hot selection matrices, bf16:
    # S[core, q, 128, col*BLK + d] = 1 iff edge at (partition, col) has
    # dst-row d within its block. Padding positions stay all-zero.
    S_np = np.zeros((NC, cfg.NQ, cfg.COLS_Q, 128, cfg.BLK), dtype=NP_BF16)
    S_np[es_c, eq_c, pos // 128, pos % 128, e_r[order2]] = 1.0
    S_dev = np.ascontiguousarray(
        S_np.transpose(0, 1, 3, 2, 4).reshape(
            NC, cfg.NQ, 128, cfg.COLS_Q * cfg.BLK))
    del S_np

    # --- per-slot node data ---
    valid = node_of_slot >= 0
    xe = np.zeros((cfg.TABLE_N, cfg.IN_FEAT), dtype=np.float32)
    xe[valid] = x[node_of_slot[valid]]
    dinv_s = np.zeros(cfg.TABLE_N, dtype=np.float32)
    dinv_s[valid] = dinv[node_of_slot[valid]]

    W1 = np.asarray(W1, np.float32)
    W2 = np.asarray(W2, np.float32).astype(NP_BF16)
    Wl = np.asarray(Wl, np.float32).astype(NP_BF16)
    b1 = np.asarray(b1, np.float32)
    b2 = np.asarray(b2, np.float32)
    bl = np.asarray(bl, np.float32)

    ident2 = np.concatenate([np.eye(cfg.HIDDEN), np.eye(cfg.HIDDEN)],
                            axis=0).astype(cfg.NP_SELF)

    in_maps = []
    for c in range(NC):
        sl = slice(c * cfg.SLOTS_C, (c + 1) * cfg.SLOTS_C)
        dv = dinv_s[sl]
        m = {
            "xT": np.ascontiguousarray(xe[sl].T),
            "w1": W1, "w2": W2, "wl": Wl,
            "b1c": b1.reshape(-1, 1), "b2c": b2.reshape(-1, 1),
            "blrep": np.tile(bl[None, :], (128, 1)),
            "dinvn": np.ascontiguousarray(dv.reshape(cfg.NT, 128).T),
            "dinvfm": np.tile(dv[None, :], (cfg.HIDDEN, 1)),
            "ident2": ident2,
        }
        for q in range(cfg.NQ):
            m[f"idx{q}"] = idx_dev[c, q]
            m[f"S{q}"] = S_dev[c, q]
        in_maps.append(m)

    return in_maps, node_of_slot


def assemble_output(cfg, results, node_of_slot):
    out = np.zeros((cfg.N, cfg.N_CLASSES), dtype=np.float32)
    for c, r in enumerate(results):
        lg = r["logits"].reshape(128, cfg.NT, cfg.N_CLASSES)
        sl = node_of_slot[c * cfg.SLOTS_C:(c + 1) * cfg.SLOTS_C]\
            .reshape(cfg.NT, 128)
        for t in range(cfg.NT):
            v = sl[t] >= 0
            out[sl[t][v]] = lg[v, t, :]
    return out


# ---------------------------------------------------------------------------
# Device program
# ---------------------------------------------------------------------------

def build_program(cfg):
    nc = bacc.Bacc("TRN2", target_bir_lowering=False, debug=False,
                   num_devices=cfg.NC, num_swdge_queues=4)
    H, NT = cfg.HIDDEN, cfg.NT

    xT_d = nc.dram_tensor("xT", [cfg.IN_FEAT, cfg.SLOTS_C], F32,
                          kind="ExternalInput")
    w1_d = nc.dram_tensor("w1", [cfg.IN_FEAT, H], F32, kind="ExternalInput")
    w2_d = nc.dram_tensor("w2", [H, H], BF16, kind="ExternalInput")
    wl_d = nc.dram_tensor("wl", [H, cfg.N_CLASSES], BF16,
                          kind="ExternalInput")
    b1c_d = nc.dram_tensor("b1c", [H, 1], F32, kind="ExternalInput")
    b2c_d = nc.dram_tensor("b2c", [H, 1], F32, kind="ExternalInput")
    blrep_d = nc.dram_tensor("blrep", [128, cfg.N_CLASSES], F32,
                             kind="ExternalInput")
    dinvn_d = nc.dram_tensor("dinvn", [128, NT], F32, kind="ExternalInput")
    dinvfm_d = nc.dram_tensor("dinvfm", [H, cfg.SLOTS_C], F32,
                              kind="ExternalInput")
    ident_d = nc.dram_tensor("ident2", [128, H], cfg.SELF_DT,
                             kind="ExternalInput")
    idx_d = [nc.dram_tensor(f"idx{q}", [128, cfg.COLS_Q * 8], I16,
                            kind="ExternalInput") for q in range(cfg.NQ)]
    S_d = [nc.dram_tensor(f"S{q}", [128, cfg.COLS_Q * cfg.BLK], BF16,
                          kind="ExternalInput") for q in range(cfg.NQ)]
    logits_d = nc.dram_tensor("logits", [128, NT * cfg.N_CLASSES], F32,
                              kind="ExternalOutput")

    rg = [list(range(cfg.NC))]

    with tile.TileContext(nc) as tc:
        with tc.tile_pool(name="const", bufs=1) as cpool, \
             tc.tile_pool(name="dram", bufs=1, space="DRAM") as dpool, \
             tc.tile_pool(name="hp", bufs=3) as hpool:

            # hs chunk tiles: collective k fires as soon as its quarter of
            # the local shard is written, overlapping the producing layer.
            NCHUNK = 4
            CH = cfg.SLOTS_C // NCHUNK
            TPC = NT // NCHUNK            # 128-row tiles per chunk
            hs1_c = [dpool.tile([CH, H], BF16, tag=f"hs1c{k}",
                                name=f"hs1c{k}") for k in range(NCHUNK)]
            hs2_c = [dpool.tile([CH, H], BF16, tag=f"hs2c{k}",
                                name=f"hs2c{k}") for k in range(NCHUNK)]
            tab1_t = dpool.tile([cfg.TABLE_N, H], BF16, tag="tab1",
                                name="tab1_t")
            tab2_t = dpool.tile([cfg.TABLE_N, H], BF16, tag="tab2",
                                name="tab2_t")
            # chunk k of the table = contiguous rows (stripe-major layout)
            CHT = cfg.TABLE_N // NCHUNK
            tab1_v = [tab1_t[k * CHT:(k + 1) * CHT, :]
                      for k in range(NCHUNK)]
            tab2_v = [tab2_t[k * CHT:(k + 1) * CHT, :]
                      for k in range(NCHUNK)]

            def cload(dram, shape, dt, tag):
                t = cpool.tile(shape, dt, tag=tag)
                nc.sync.dma_start(out=t[:], in_=dram[:, :])
                return t

            w1_s = cload(w1_d, [cfg.IN_FEAT, H], F32, "w1")
            w2_s = cload(w2_d, [H, H], BF16, "w2")
            wl_s = cload(wl_d, [H, cfg.N_CLASSES], BF16, "wl")
            b1c_s = cload(b1c_d, [H, 1], F32, "b1c")
            b2c_s = cload(b2c_d, [H, 1], F32, "b2c")
            blrep_s = cload(blrep_d, [128, cfg.N_CLASSES], F32, "blrep")
            dinvn_s = cload(dinvn_d, [128, NT], F32, "dinvn")
            ident_s = cload(ident_d, [128, H], cfg.SELF_DT, "ident")

            self1_s = cpool.tile([128, NT * H], cfg.SELF_DT, tag="self1")
            self2_s = cpool.tile([128, NT * H], cfg.SELF_DT, tag="self2")
            stageL_s = cpool.tile([128, NT * cfg.N_CLASSES], F32, tag="stgL")

            # ---- phase A: table1 = dinv * (x @ W1), plus self terms ----
            with tc.tile_pool(name="xp", bufs=1) as xpool, \
                 tc.tile_pool(name="pA", bufs=2, space="PSUM") as pA:
                xt_s = xpool.tile([cfg.IN_FEAT, cfg.SLOTS_C], F32, tag="xt")
                for k in range(NCHUNK):
                    nc.sync.dma_start(out=xt_s[:, k * CH:(k + 1) * CH],
                                      in_=xT_d[:, k * CH:(k + 1) * CH])
                for t in range(NT):
                    ps = pA.tile([128, H], F32, tag="a")
                    nc.tensor.matmul(out=ps[:],
                                     lhsT=xt_s[:, t * 128:(t + 1) * 128],
                                     rhs=w1_s[:], start=True, stop=True)
                    row = hpool.tile([128, H], BF16, tag="hsrow")
                    nc.vector.tensor_scalar_mul(out=row[:], in0=ps[:],
                                                scalar1=dinvn_s[:, t:t + 1])
                    k, tk = t // TPC, t % TPC
                    nc.sync.dma_start(
                        out=hs1_c[k][tk * 128:(tk + 1) * 128, :], in_=row[:])
                    nc.vector.tensor_scalar_mul(
                        out=self1_s[:, t * H:(t + 1) * H], in0=ps[:],
                        scalar1=dinvn_s[:, t:t + 1])
                    if tk == TPC - 1:
                        nc.gpsimd.collective_compute(
                            "AllGather", mybir.AluOpType.bypass,
                            replica_groups=rg, ins=[hs1_c[k][:, :]],
                            outs=[tab1_v[k]])

            # ---- phases B (layer1 -> table2) and C (layer2 -> logits) ----
            with tc.tile_pool(name="sp", bufs=3) as spool, \
                 tc.tile_pool(name="pp", bufs=2, space="PSUM") as pp:

                def conv_layer(layer):
                    tab_t = tab1_t if layer == 1 else tab2_t
                    self_s = self1_s if layer == 1 else self2_s
                    bc_s = b1c_s if layer == 1 else b2c_s
                    # paired-row view of the table: one 256B gather element
                    # covers two consecutive bf16 rows (slots 2w, 2w+1)
                    tabp = tab_t[:].rearrange("(n two) h -> n (two h)", two=2)
                    pair = {}
                    fired = [False] * NCHUNK

                    def fire_tab2(k):
                        nc.gpsimd.collective_compute(
                            "AllGather", mybir.AluOpType.bypass,
                            replica_groups=rg, ins=[hs2_c[k][:, :]],
                            outs=[tab2_v[k]])
                        fired[k] = True

                    for i in range(cfg.N_BATCH):
                        msgs, Ss = [None] * cfg.NQ, [None] * cfg.NQ
                        for q in (list(range(1, cfg.NQ)) + [0]):
                            idx_t = spool.tile([128, cfg.C_BATCH * 8], I16,
                                               tag=f"idx{q}")
                            nc.sync.dma_start(
                                out=idx_t[:],
                                in_=idx_d[q][:, i * cfg.C_BATCH * 8:
                                             (i + 1) * cfg.C_BATCH * 8])
                            msg_t = spool.tile([128, cfg.C_BATCH, 2 * H],
                                               BF16, tag=f"msg{q}")
                            nc.gpsimd.dma_gather(
                                out_ap=msg_t[:],
                                in_ap=tabp[q * cfg.WIN // 2:
                                           (q + 1) * cfg.WIN // 2, :],
                                idxs_ap=idx_t[:],
                                num_idxs=cfg.C_BATCH * 128,
                                num_idxs_reg=cfg.C_BATCH * 128,
                                elem_size=2 * H, queue_num=q,
                                single_packet=False)
                            msgs[q] = msg_t[:].rearrange("p c f -> p (c f)")
                        if layer == 1:
                            # fire chunk collectives two batches after their
                            # last row landed, AFTER this batch's gathers, so
                            # the (in-order) Pool trigger never blocks them
                            for k in range(NCHUNK):
                                if not fired[k] and i >= 4 * k + 7:
                                    fire_tab2(k)
                        for q in range(cfg.NQ):
                            S_t = spool.tile([128, cfg.C_BATCH * cfg.BLK],
                                             BF16, tag=f"S{q}")
                            nc.sync.dma_start(
                                out=S_t[:],
                                in_=S_d[q][:, i * cfg.C_BATCH * cfg.BLK:
                                           (i + 1) * cfg.C_BATCH * cfg.BLK])
                            Ss[q] = S_t[:]

                        dfm_t = spool.tile([H, cfg.BPB * cfg.BLK], F32,
                                           tag="dfm")
                        nc.sync.dma_start(
                            out=dfm_t[:],
                            in_=dinvfm_d[:, i * cfg.BPB * cfg.BLK:
                                         (i + 1) * cfg.BPB * cfg.BLK])

                        for bb in range(cfg.BPB):
                            b = i * cfg.BPB + bb
                            half = (b % 2) * H
                            t = b // 2
                            pfm = pp.tile([H, cfg.BLK], F32, tag="fm")
                            nc.tensor.matmul(
                                out=pfm[:],
                                lhsT=self_s[half:half + H,
                                            t * H:(t + 1) * H],
                                rhs=ident_s[half:half + H, :],
                                start=True, stop=False)
                            for q in range(cfg.NQ):
                                for k in range(cfg.KCOL):
                                    lc = bb * cfg.KCOL + k
                                    last = (q == cfg.NQ - 1 and
                                            k == cfg.KCOL - 1)
                                    # column parity k selects the half of
                                    # the gathered pair element
                                    nc.tensor.matmul(
                                        out=pfm[:],
                                        lhsT=msgs[q][:, lc * 2 * H + k * H:
                                                     lc * 2 * H +
                                                     (k + 1) * H],
                                        rhs=Ss[q][:, lc * cfg.BLK:
                                                  (lc + 1) * cfg.BLK],
                                        start=False, stop=last)
                            h_t = hpool.tile([H, cfg.BLK], F32, tag="h")
                            nc.vector.tensor_tensor(
                                out=h_t[:], in0=pfm[:],
                                in1=dfm_t[:, bb * cfg.BLK:(bb + 1) * cfg.BLK],
                                op=mybir.AluOpType.mult)
                            hr_t = hpool.tile([H, cfg.BLK], BF16, tag="hr")
                            nc.scalar.activation(
                                out=hr_t[:], in_=h_t[:],
                                func=mybir.ActivationFunctionType.Relu,
                                bias=bc_s[:])
                            if layer == 1:
                                if b % 2 == 0:
                                    pair["p2"] = pp.tile([128, H], F32, name="p2",
                                                         tag="pair")
                                p2 = pair["p2"]
                                nc.tensor.matmul(
                                    out=p2[half:half + H, :], lhsT=hr_t[:],
                                    rhs=w2_s[:], start=True, stop=True,
                                    tile_position=(0, half))
                                if b % 2 == 1:
                                    row2 = hpool.tile([128, H], BF16,
                                                      tag="hs2row")
                                    nc.vector.tensor_scalar_mul(
                                        out=row2[:], in0=p2[:],
                                        scalar1=dinvn_s[:, t:t + 1])
                                    k, tk = t // TPC, t % TPC
                                    nc.sync.dma_start(
                                        out=hs2_c[k][tk * 128:
                                                     (tk + 1) * 128, :],
                                        in_=row2[:])
                                    nc.vector.tensor_scalar_mul(
                                        out=self2_s[:, t * H:(t + 1) * H],
                                        in0=p2[:],
                                        scalar1=dinvn_s[:, t:t + 1])
                            else:
                                if b % 2 == 0:
                                    pair["pl"] = pp.tile([128, cfg.N_CLASSES],
                                                         F32, name="pl", tag="pl")
                                pl = pair["pl"]
                                nc.tensor.matmul(
                                    out=pl[half:half + H, :], lhsT=hr_t[:],
                                    rhs=wl_s[:], start=True, stop=True,
                                    tile_position=(0, half))
                                if b % 2 == 1:
                                    nCL = cfg.N_CLASSES
                                    nc.vector.tensor_tensor(
                                        out=stageL_s[:, t * nCL:(t + 1) * nCL],
                                        in0=pl[:], in1=blrep_s[:],
                                        op=mybir.AluOpType.add)

                    if layer == 1:
                        for k in range(NCHUNK):
                            if not fired[k]:
                                fire_tab2(k)

                conv_layer(1)
                conv_layer(2)

            nc.sync.dma_start(out=logits_d[:, :], in_=stageL_s[:])

    nc.compile()
    return nc


_PROGRAM_CACHE = {}


def get_program(cfg):
    key = id(cfg)
    if key not in _PROGRAM_CACHE:
        _PROGRAM_CACHE[key] = build_program(cfg)
    return _PROGRAM_CACHE[key]


def run(cfg, inputs, trace=False):
    in_maps, node_of_slot = preprocess(cfg, **inputs)
    nc = get_program(cfg)
    res = bass_utils.run_bass_kernel_spmd(
        nc, in_maps, core_ids=list(range(cfg.NC)), trace=trace)
    out = assemble_output(cfg, res.results, node_of_slot)
    return out, res


def kernel(**inputs) -> np.ndarray:
    out, _ = run(CFG_FULL, inputs)
    return out



# revision 42
# speedup vs baseline: 1.0124x; 1.0124x over previous
"""2-layer GCN (GCNConv+relu x2, linear head) on 8 Trainium2 NeuronCores.

Strategy (graph/data parallel, per sharding hint):
  - Nodes sharded across 8 cores by id; edges partitioned by destination.
  - Per core, destination nodes are bin-packed into B_FIX blocks of <=BLK
    dsts such that each (block, source-window) holds <= KCOL*128 edges.
    This gives an SPMD-uniform program; only tensor data varies per core.
  - Per layer: local matmul (x@W scaled by dinv) -> AllGather into a
    full node-major bf16 table in DRAM -> dma_gather one 256B element
    per edge = a PAIR of bf16 rows (slots 2w, 2w+1); parity-split
    selection matrices S_even/S_odd (is_equal on DVE, bf16) route the
    correct half; PE bf16 matmuls accumulate feature-major conv output
    in PSUM; self-loop terms enter via an identity-matmul transpose.
    Post: scale by dinv, +bias, relu, next-layer matmul (bf16).
  - The 4 source-window gathers go to 4 SWDGE queues: each queue's
    descriptor generation runs on its own GpSimd Q7 core pair, so the
    4 gathers of a batch overlap (queue 0 issued last since its pair
    is the one the engine timeline blocks on).
"""

import numpy as np

import concourse.bass as bass
import concourse.mybir as mybir
import concourse.tile as tile
from concourse import bacc
from concourse import bass_utils

import ml_dtypes

F32 = mybir.dt.float32
BF16 = mybir.dt.bfloat16
I16 = mybir.dt.int16
NP_BF16 = ml_dtypes.bfloat16


class Cfg:
    def __init__(self, n_nodes, in_feat, hidden, n_classes, n_cores, n_c,
                 blk, kcol, b_fix, nq, c_batch, self_dtype="bf16"):
        self.N = n_nodes
        self.IN_FEAT = in_feat
        self.HIDDEN = hidden
        self.N_CLASSES = n_classes
        self.NC = n_cores
        self.N_C = n_c                    # nodes per core (id // N_C)
        assert n_c * n_cores >= n_nodes
        self.BLK = blk                    # max dsts per block
        self.KCOL = kcol                  # columns per (block, stream)
        self.CAP = kcol * 128             # max edges per (block, stream)
        self.B_FIX = b_fix                # blocks per core (uniform)
        self.NQ = nq                      # source windows / gather streams
        self.SLOTS_C = b_fix * blk        # table slots per core
        assert self.SLOTS_C % 128 == 0
        self.NT = self.SLOTS_C // 128     # node tiles per core
        assert self.NT % 2 == 0
        self.TABLE_N = n_cores * self.SLOTS_C
        assert self.TABLE_N % nq == 0
        self.WIN = self.TABLE_N // nq     # table rows per source window
        assert self.WIN <= 32767          # int16 gather index range
        assert (n_cores % nq) == 0
        self.COLS_Q = b_fix * kcol        # gather columns per stream
        self.C_BATCH = c_batch            # columns per gather batch
        assert c_batch % kcol == 0 and self.COLS_Q % c_batch == 0
        self.N_BATCH = self.COLS_Q // c_batch
        self.BPB = c_batch // kcol        # blocks per batch
        assert self.BPB % 2 == 0          # block pairs never straddle batches
        self.SELF_DT = BF16 if self_dtype == "bf16" else F32
        self.NP_SELF = NP_BF16 if self_dtype == "bf16" else np.float32
        self.NCHUNK = 8                   # AllGather chunks per layer
        assert self.NCHUNK % nq == 0 and self.NT % self.NCHUNK == 0


CFG_FULL = Cfg(n_nodes=100000, in_feat=128, hidden=64, n_classes=16,
               n_cores=8, n_c=12544, blk=64, kcol=2, b_fix=224, nq=4,
               c_batch=28)


# ---------------------------------------------------------------------------
# Host-side preprocessing (sharding): all integer graph restructuring.
# ---------------------------------------------------------------------------

def preprocess(cfg, x, edge_index, W1, b1, W2, b2, Wl, bl):
    N, NC, N_C = cfg.N, cfg.NC, cfg.N_C
    src = np.asarray(edge_index[0]).astype(np.int64)
    dst = np.asarray(edge_index[1]).astype(np.int64)
    x = np.asarray(x, dtype=np.float32)

    deg = np.bincount(dst, minlength=N).astype(np.float32) + 1.0
    dinv = (1.0 / np.sqrt(deg)).astype(np.float32)

    # stream = stripe of the SOURCE node, chosen a priori and independent
    # of its parity class: stripe k nodes get packed into blocks
    # [56k, 56(k+1)) of their core, so table window q is the contiguous
    # rows [q*TABLE_N/NQ, ...) written by the q-th chunked AllGather.
    q_of = ((src // 2) % cfg.NQ).astype(np.int64)
    p_of = (src % 2).astype(np.int64)        # a-priori source parity class

    # per-(node, q, par) incoming edge counts
    degqp = np.bincount((dst * cfg.NQ + q_of) * 2 + p_of,
                        minlength=N * cfg.NQ * 2).reshape(N, cfg.NQ, 2)

    # --- per-core first-fit-decreasing packing of dsts into blocks ---
    # Constraints: per (block, q, par) <= 128 edges (one 128-row column);
    # node with id parity p gets a block position r with r % 2 == p, so
    # slot parity == id parity (known before packing any core).
    HBLK = cfg.BLK // 2
    B_STR = cfg.B_FIX // cfg.NQ          # blocks per stripe
    node_q = ((np.arange(NC * N_C) // 2) % cfg.NQ)
    slot_of = np.full(NC * N_C, -1, dtype=np.int64)
    node_of_slot = np.full(cfg.TABLE_N, -1, dtype=np.int64)
    for c in range(NC):
        lo, hi = c * N_C, min((c + 1) * N_C, N)
        if hi <= lo:
            continue
        for k in range(cfg.NQ):
            ids = lo + np.flatnonzero(node_q[lo:hi] == k)
            dq = degqp[ids].reshape(ids.size, cfg.NQ * 2)
            order = np.argsort(-dq.max(axis=1), kind="stable")
            accs = np.zeros((B_STR, cfg.NQ * 2), dtype=np.int64)
            cnts = np.zeros((B_STR, 2), dtype=np.int64)
            nopen = 1
            for j in order:
                v = dq[j]
                g = int(ids[j])
                pj = g % 2
                fits = (cnts[:nopen, pj] < HBLK) & \
                       np.all(accs[:nopen] + v <= 128, axis=1)
                w = np.flatnonzero(fits)
                if w.size == 0:
                    assert nopen < B_STR, \
                        f"core {c} stripe {k}: packing exceeds {B_STR} blocks"
                    b = nopen
                    nopen += 1
                else:
                    b = int(w[0])
                r = 2 * cnts[b, pj] + pj
                s = c * cfg.SLOTS_C + (k * B_STR + b) * cfg.BLK + r
                slot_of[g] = s
                node_of_slot[s] = g
                accs[b] += v
                cnts[b, pj] += 1

    slot_of = slot_of[:N]

    # --- per-core edge streams ---
    e_core = dst // N_C
    s_slot = slot_of[src]
    d_slot_l = slot_of[dst] - e_core * cfg.SLOTS_C
    e_b = d_slot_l // cfg.BLK
    e_r = d_slot_l % cfg.BLK

    P_Q = cfg.B_FIX * cfg.CAP            # positions per stream
    idx_all = np.zeros((NC, cfg.NQ, P_Q), dtype=np.int16)

    e_par = (s_slot % 2).astype(np.int64)    # == src % 2 by construction
    order2 = np.lexsort((e_par, e_b, q_of, e_core))
    es_c, eq_c, eb_c = e_core[order2], q_of[order2], e_b[order2]
    ep_c = e_par[order2]
    grp = ((es_c * cfg.NQ + eq_c) * cfg.B_FIX + eb_c) * 2 + ep_c
    _, start_idx, cnt_grp = np.unique(grp, return_index=True,
                                      return_counts=True)
    rank = np.arange(grp.size) - np.repeat(start_idx, cnt_grp)
    assert rank.max(initial=0) < 128
    # column = 2*block + parity; position = column*128 + rank
    pos = eb_c * cfg.CAP + ep_c * 128 + rank
    s_sorted = s_slot[order2]
    # Table layout is chunk-major: chunk k (k = local_slot // CHR, CHR =
    # SLOTS_C/NCHUNK) holds every core's rows [c*CHR, (c+1)*CHR).
    # Window q = chunks [q*CPW, (q+1)*CPW) is still contiguous.
    STR_ROWS = cfg.SLOTS_C // cfg.NQ
    CHR = cfg.SLOTS_C // cfg.NCHUNK
    CPW = cfg.NCHUNK // cfg.NQ           # chunks per window
    l_sorted = s_sorted % cfg.SLOTS_C
    assert np.all(l_sorted // STR_ROWS == eq_c)
    k_l = l_sorted // CHR
    wrow = (k_l % CPW) * (cfg.NC * CHR) + \
        (s_sorted // cfg.SLOTS_C) * CHR + (l_sorted % CHR)
    # gather PAIR index (two table rows per 256B element)
    idx_val = (wrow // 2).astype(np.int16)
    idx_all[es_c, eq_c, pos] = idx_val

    # wrapped int16 layout: position i -> [i%16, i//16], replicated x8
    idx_w = idx_all.reshape(NC, cfg.NQ, -1, 16).transpose(0, 1, 3, 2)
    idx_dev = np.ascontiguousarray(np.tile(idx_w, (1, 1, 8, 1)))

    # host-built one-hot selection matrices, bf16:
    # S[core, q, 128, col*BLK + d] = 1 iff edge at (partition, col) has
    # dst-row d within its block. Padding positions stay all-zero.
    S_np = np.zeros((NC, cfg.NQ, cfg.COLS_Q, 128, cfg.BLK), dtype=NP_BF16)
    S_np[es_c, eq_c, pos // 128, pos % 128, e_r[order2]] = 1.0
    S_dev = np.ascontiguousarray(
        S_np.transpose(0, 1, 3, 2, 4).reshape(
            NC, cfg.NQ, 128, cfg.COLS_Q * cfg.BLK))
    del S_np

    # --- per-slot node data ---
    valid = node_of_slot >= 0
    xe = np.zeros((cfg.TABLE_N, cfg.IN_FEAT), dtype=np.float32)
    xe[valid] = x[node_of_slot[valid]]
    dinv_s = np.zeros(cfg.TABLE_N, dtype=np.float32)
    dinv_s[valid] = dinv[node_of_slot[valid]]

    W1 = np.asarray(W1, np.float32)
    W2 = np.asarray(W2, np.float32).astype(NP_BF16)
    Wl = np.asarray(Wl, np.float32).astype(NP_BF16)
    b1 = np.asarray(b1, np.float32)
    b2 = np.asarray(b2, np.float32)
    bl = np.asarray(bl, np.float32)

    ident2 = np.concatenate([np.eye(cfg.HIDDEN), np.eye(cfg.HIDDEN)],
                            axis=0).astype(cfg.NP_SELF)

    in_maps = []
    for c in range(NC):
        sl = slice(c * cfg.SLOTS_C, (c + 1) * cfg.SLOTS_C)
        dv = dinv_s[sl]
        m = {
            "xT": np.ascontiguousarray(xe[sl].T),
            "w1": W1, "w2": W2, "wl": Wl,
            "b1c": b1.reshape(-1, 1), "b2c": b2.reshape(-1, 1),
            "blrep": np.tile(bl[None, :], (128, 1)),
            "dinvn": np.ascontiguousarray(dv.reshape(cfg.NT, 128).T),
            "dinvfm": np.tile(dv[None, :], (cfg.HIDDEN, 1)),
            "ident2": ident2,
        }
        for q in range(cfg.NQ):
            m[f"idx{q}"] = idx_dev[c, q]
            m[f"S{q}"] = S_dev[c, q]
        in_maps.append(m)

    return in_maps, node_of_slot


def assemble_output(cfg, results, node_of_slot):
    out = np.zeros((cfg.N, cfg.N_CLASSES), dtype=np.float32)
    for c, r in enumerate(results):
        lg = r["logits"].reshape(128, cfg.NT, cfg.N_CLASSES)
        sl = node_of_slot[c * cfg.SLOTS_C:(c + 1) * cfg.SLOTS_C]\
            .reshape(cfg.NT, 128)
        for t in range(cfg.NT):
            v = sl[t] >= 0
            out[sl[t][v]] = lg[v, t, :]
    return out


# ---------------------------------------------------------------------------
# Device program
# ---------------------------------------------------------------------------

def build_program(cfg):
    nc = bacc.Bacc("TRN2", target_bir_lowering=False, debug=False,
                   num_devices=cfg.NC, num_swdge_queues=4)
    H, NT = cfg.HIDDEN, cfg.NT

    xT_d = nc.dram_tensor("xT", [cfg.IN_FEAT, cfg.SLOTS_C], F32,
                          kind="ExternalInput")
    w1_d = nc.dram_tensor("w1", [cfg.IN_FEAT, H], F32, kind="ExternalInput")
    w2_d = nc.dram_tensor("w2", [H, H], BF16, kind="ExternalInput")
    wl_d = nc.dram_tensor("wl", [H, cfg.N_CLASSES], BF16,
                          kind="ExternalInput")
    b1c_d = nc.dram_tensor("b1c", [H, 1], F32, kind="ExternalInput")
    b2c_d = nc.dram_tensor("b2c", [H, 1], F32, kind="ExternalInput")
    blrep_d = nc.dram_tensor("blrep", [128, cfg.N_CLASSES], F32,
                             kind="ExternalInput")
    dinvn_d = nc.dram_tensor("dinvn", [128, NT], F32, kind="ExternalInput")
    dinvfm_d = nc.dram_tensor("dinvfm", [H, cfg.SLOTS_C], F32,
                              kind="ExternalInput")
    ident_d = nc.dram_tensor("ident2", [128, H], cfg.SELF_DT,
                             kind="ExternalInput")
    idx_d = [nc.dram_tensor(f"idx{q}", [128, cfg.COLS_Q * 8], I16,
                            kind="ExternalInput") for q in range(cfg.NQ)]
    S_d = [nc.dram_tensor(f"S{q}", [128, cfg.COLS_Q * cfg.BLK], BF16,
                          kind="ExternalInput") for q in range(cfg.NQ)]
    logits_d = nc.dram_tensor("logits", [128, NT * cfg.N_CLASSES], F32,
                              kind="ExternalOutput")

    rg = [list(range(cfg.NC))]

    with tile.TileContext(nc) as tc:
        with tc.tile_pool(name="const", bufs=1) as cpool, \
             tc.tile_pool(name="dram", bufs=1, space="DRAM") as dpool, \
             tc.tile_pool(name="hp", bufs=3) as hpool:

            # hs chunk tiles: collective k fires as soon as its slice of
            # the local shard is written, overlapping the producing layer.
            NCHUNK = cfg.NCHUNK
            CH = cfg.SLOTS_C // NCHUNK
            TPC = NT // NCHUNK            # 128-row tiles per chunk
            hs1_c = [dpool.tile([CH, H], BF16, tag=f"hs1c{k}",
                                name=f"hs1c{k}") for k in range(NCHUNK)]
            hs2_c = [dpool.tile([CH, H], BF16, tag=f"hs2c{k}",
                                name=f"hs2c{k}") for k in range(NCHUNK)]
            tab1_t = dpool.tile([cfg.TABLE_N, H], BF16, tag="tab1",
                                name="tab1_t")
            tab2_t = dpool.tile([cfg.TABLE_N, H], BF16, tag="tab2",
                                name="tab2_t")
            # chunk k of the table = contiguous rows (stripe-major layout)
            CHT = cfg.TABLE_N // NCHUNK
            tab1_v = [tab1_t[k * CHT:(k + 1) * CHT, :]
                      for k in range(NCHUNK)]
            tab2_v = [tab2_t[k * CHT:(k + 1) * CHT, :]
                      for k in range(NCHUNK)]

            def cload(dram, shape, dt, tag):
                t = cpool.tile(shape, dt, tag=tag)
                nc.sync.dma_start(out=t[:], in_=dram[:, :])
                return t

            w1_s = cload(w1_d, [cfg.IN_FEAT, H], F32, "w1")
            w2_s = cload(w2_d, [H, H], BF16, "w2")
            wl_s = cload(wl_d, [H, cfg.N_CLASSES], BF16, "wl")
            b1c_s = cload(b1c_d, [H, 1], F32, "b1c")
            b2c_s = cload(b2c_d, [H, 1], F32, "b2c")
            blrep_s = cload(blrep_d, [128, cfg.N_CLASSES], F32, "blrep")
            dinvn_s = cload(dinvn_d, [128, NT], F32, "dinvn")
            ident_s = cload(ident_d, [128, H], cfg.SELF_DT, "ident")

            self1_s = cpool.tile([128, NT * H], cfg.SELF_DT, tag="self1")
            self2_s = cpool.tile([128, NT * H], cfg.SELF_DT, tag="self2")
            stageL_s = cpool.tile([128, NT * cfg.N_CLASSES], F32, tag="stgL")

            # ---- phase A: table1 = dinv * (x @ W1), plus self terms ----
            with tc.tile_pool(name="xp", bufs=1) as xpool, \
                 tc.tile_pool(name="pA", bufs=2, space="PSUM") as pA:
                xt_s = xpool.tile([cfg.IN_FEAT, cfg.SLOTS_C], F32, tag="xt")
                for k in range(NCHUNK):
                    nc.sync.dma_start(out=xt_s[:, k * CH:(k + 1) * CH],
                                      in_=xT_d[:, k * CH:(k + 1) * CH])
                for t in range(NT):
                    ps = pA.tile([128, H], F32, tag="a")
                    nc.tensor.matmul(out=ps[:],
                                     lhsT=xt_s[:, t * 128:(t + 1) * 128],
                                     rhs=w1_s[:], start=True, stop=True)
                    row = hpool.tile([128, H], BF16, tag="hsrow")
                    nc.vector.tensor_scalar_mul(out=row[:], in0=ps[:],
                                                scalar1=dinvn_s[:, t:t + 1])
                    k, tk = t // TPC, t % TPC
                    nc.sync.dma_start(
                        out=hs1_c[k][tk * 128:(tk + 1) * 128, :], in_=row[:])
                    nc.vector.tensor_scalar_mul(
                        out=self1_s[:, t * H:(t + 1) * H], in0=ps[:],
                        scalar1=dinvn_s[:, t:t + 1])
                    if tk == TPC - 1:
                        nc.gpsimd.collective_compute(
                            "AllGather", mybir.AluOpType.bypass,
                            replica_groups=rg, ins=[hs1_c[k][:, :]],
                            outs=[tab1_v[k]])

            # ---- phases B (layer1 -> table2) and C (layer2 -> logits) ----
            with tc.tile_pool(name="sp", bufs=3) as spool, \
                 tc.tile_pool(name="pp", bufs=2, space="PSUM") as pp:

                def conv_layer(layer):
                    tab_t = tab1_t if layer == 1 else tab2_t
                    self_s = self1_s if layer == 1 else self2_s
                    bc_s = b1c_s if layer == 1 else b2c_s
                    # paired-row view of the table: one 256B gather element
                    # covers two consecutive bf16 rows (slots 2w, 2w+1)
                    tabp = tab_t[:].rearrange("(n two) h -> n (two h)", two=2)
                    pair = {}
                    fired = [False] * NCHUNK

                    def fire_tab2(k):
                        nc.gpsimd.collective_compute(
                            "AllGather", mybir.AluOpType.bypass,
                            replica_groups=rg, ins=[hs2_c[k][:, :]],
                            outs=[tab2_v[k]])
                        fired[k] = True

                    # layer 1 runs batches 4..15 first so stream 0's table
                    # chunks (written by batches 0..3) land last — its
                    # layer-2 gather is the one issued last per batch, so
                    # the remaining collective latency hides behind the
                    # other streams' descriptor generation.
                    border = (list(range(4, cfg.N_BATCH)) + list(range(4))
                              if layer == 1 else list(range(cfg.N_BATCH)))
                    BPC = cfg.N_BATCH // NCHUNK
                    done_pos = [max(border.index(b)
                                    for b in range(k * BPC, (k + 1) * BPC))
                                for k in range(NCHUNK)]

                    for j in range(cfg.N_BATCH):
                        i = border[j]
                        msgs, Ss = [None] * cfg.NQ, [None] * cfg.NQ
                        for q in (list(range(1, cfg.NQ)) + [0]):
                            idx_t = spool.tile([128, cfg.C_BATCH * 8], I16,
                                               tag=f"idx{q}")
                            nc.sync.dma_start(
                                out=idx_t[:],
                                in_=idx_d[q][:, i * cfg.C_BATCH * 8:
                                             (i + 1) * cfg.C_BATCH * 8])
                            msg_t = spool.tile([128, cfg.C_BATCH, 2 * H],
                                               BF16, tag=f"msg{q}")
                            nc.gpsimd.dma_gather(
                                out_ap=msg_t[:],
                                in_ap=tabp[q * cfg.WIN // 2:
                                           (q + 1) * cfg.WIN // 2, :],
                                idxs_ap=idx_t[:],
                                num_idxs=cfg.C_BATCH * 128,
                                num_idxs_reg=cfg.C_BATCH * 128,
                                elem_size=2 * H, queue_num=q,
                                single_packet=False)
                            msgs[q] = msg_t[:].rearrange("p c f -> p (c f)")
                        if layer == 1:
                            # fire chunk collectives two processed batches
                            # after their last row landed, AFTER this batch's
                            # gathers, so the (in-order) Pool trigger never
                            # blocks them
                            for k in range(NCHUNK):
                                if not fired[k] and j >= done_pos[k] + 3:
                                    fire_tab2(k)
                        for q in range(cfg.NQ):
                            S_t = spool.tile([128, cfg.C_BATCH * cfg.BLK],
                                             BF16, tag=f"S{q}")
                            # ACT-ring HWDGE: keeps the big S loads off the
                            # Sync sequencer, which handles everything else
                            nc.scalar.dma_start(
                                out=S_t[:],
                                in_=S_d[q][:, i * cfg.C_BATCH * cfg.BLK:
                                           (i + 1) * cfg.C_BATCH * cfg.BLK])
                            Ss[q] = S_t[:]

                        dfm_t = spool.tile([H, cfg.BPB * cfg.BLK], F32,
                                           tag="dfm")
                        nc.sync.dma_start(
                            out=dfm_t[:],
                            in_=dinvfm_d[:, i * cfg.BPB * cfg.BLK:
                                         (i + 1) * cfg.BPB * cfg.BLK])

                        for bb in range(cfg.BPB):
                            b = i * cfg.BPB + bb
                            half = (b % 2) * H
                            t = b // 2
                            pfm = pp.tile([H, cfg.BLK], F32, tag="fm")
                            nc.tensor.matmul(
                                out=pfm[:],
                                lhsT=self_s[half:half + H,
                                            t * H:(t + 1) * H],
                                rhs=ident_s[half:half + H, :],
                                start=True, stop=False)
                            for q in range(cfg.NQ):
                                for k in range(cfg.KCOL):
                                    lc = bb * cfg.KCOL + k
                                    last = (q == cfg.NQ - 1 and
                                            k == cfg.KCOL - 1)
                                    # column parity k selects the half of
                                    # the gathered pair element
                                    nc.tensor.matmul(
                                        out=pfm[:],
                                        lhsT=msgs[q][:, lc * 2 * H + k * H:
                                                     lc * 2 * H +
                                                     (k + 1) * H],
                                        rhs=Ss[q][:, lc * cfg.BLK:
                                                  (lc + 1) * cfg.BLK],
                                        start=False, stop=last)
                            h_t = hpool.tile([H, cfg.BLK], F32, tag="h")
                            nc.vector.tensor_tensor(
                                out=h_t[:], in0=pfm[:],
                                in1=dfm_t[:, bb * cfg.BLK:(bb + 1) * cfg.BLK],
                                op=mybir.AluOpType.mult)
                            hr_t = hpool.tile([H, cfg.BLK], BF16, tag="hr")
                            nc.scalar.activation(
                                out=hr_t[:], in_=h_t[:],
                                func=mybir.ActivationFunctionType.Relu,
                                bias=bc_s[:])
                            if layer == 1:
                                if b % 2 == 0:
                                    pair["p2"] = pp.tile([128, H], F32, name="p2",
                                                         tag="pair")
                                p2 = pair["p2"]
                                nc.tensor.matmul(
                                    out=p2[half:half + H, :], lhsT=hr_t[:],
                                    rhs=w2_s[:], start=True, stop=True,
                                    tile_position=(0, half))
                                if b % 2 == 1:
                                    row2 = hpool.tile([128, H], BF16,
                                                      tag="hs2row")
                                    nc.vector.tensor_scalar_mul(
                                        out=row2[:], in0=p2[:],
                                        scalar1=dinvn_s[:, t:t + 1])
                                    k, tk = t // TPC, t % TPC
                                    nc.sync.dma_start(
                                        out=hs2_c[k][tk * 128:
                                                     (tk + 1) * 128, :],
                                        in_=row2[:])
                                    nc.vector.tensor_scalar_mul(
                                        out=self2_s[:, t * H:(t + 1) * H],
                                        in0=p2[:],
                                        scalar1=dinvn_s[:, t:t + 1])
                            else:
                                if b % 2 == 0:
                                    pair["pl"] = pp.tile([128, cfg.N_CLASSES],
                                                         F32, name="pl", tag="pl")
                                pl = pair["pl"]
                                nc.tensor.matmul(
                                    out=pl[half:half + H, :], lhsT=hr_t[:],
                                    rhs=wl_s[:], start=True, stop=True,
                                    tile_position=(0, half))
                                if b % 2 == 1:
                                    nCL = cfg.N_CLASSES
                                    nc.vector.tensor_tensor(
                                        out=stageL_s[:, t * nCL:(t + 1) * nCL],
                                        in0=pl[:], in1=blrep_s[:],
                                        op=mybir.AluOpType.add)

                    if layer == 1:
                        for k in range(NCHUNK):
                            if not fired[k]:
                                fire_tab2(k)

                conv_layer(1)
                conv_layer(2)

            nc.sync.dma_start(out=logits_d[:, :], in_=stageL_s[:])

    nc.compile()
    return nc


_PROGRAM_CACHE = {}


def get_program(cfg):
    key = id(cfg)
    if key not in _PROGRAM_CACHE:
        _PROGRAM_CACHE[key] = build_program(cfg)
    return _PROGRAM_CACHE[key]


def run(cfg, inputs, trace=False):
    in_maps, node_of_slot = preprocess(cfg, **inputs)
    nc = get_program(cfg)
    res = bass_utils.run_bass_kernel_spmd(
        nc, in_maps, core_ids=list(range(cfg.NC)), trace=trace)
    out = assemble_output(cfg, res.results, node_of_slot)
    return out, res


def kernel(**inputs) -> np.ndarray:
    out, _ = run(CFG_FULL, inputs)
    return out



# revision 67
# speedup vs baseline: 1.2783x; 1.2626x over previous
"""2-layer GCN (GCNConv+relu x2, linear head) on 8 Trainium2 NeuronCores.

Strategy (graph/data parallel, per sharding hint):
  - Nodes sharded across 8 cores by id; edges partitioned by destination.
  - Per core, destination nodes are bin-packed into B_FIX blocks of <=BLK
    dsts such that each (block, source-window) holds <= KCOL*128 edges.
    This gives an SPMD-uniform program; only tensor data varies per core.
  - Per layer: local matmul (x@W scaled by dinv) -> AllGather into a
    full node-major bf16 table in DRAM -> dma_gather one 256B element
    per edge = a PAIR of bf16 rows (slots 2w, 2w+1); parity-split
    selection matrices S_even/S_odd (is_equal on DVE, bf16) route the
    correct half; PE bf16 matmuls accumulate feature-major conv output
    in PSUM; self-loop terms enter via an identity-matmul transpose.
    Post: scale by dinv, +bias, relu, next-layer matmul (bf16).
  - The 4 source-window gathers go to 4 SWDGE queues: each queue's
    descriptor generation runs on its own GpSimd Q7 core pair, so the
    4 gathers of a batch overlap (queue 0 issued last since its pair
    is the one the engine timeline blocks on).
"""

import numpy as np

import concourse.bass as bass
import concourse.mybir as mybir
import concourse.tile as tile
from concourse import bacc
from concourse import bass_utils

import ml_dtypes

F32 = mybir.dt.float32
BF16 = mybir.dt.bfloat16
I16 = mybir.dt.int16
NP_BF16 = ml_dtypes.bfloat16


class Cfg:
    def __init__(self, n_nodes, in_feat, hidden, n_classes, n_cores, n_c,
                 blk, kcol, b_fix, nq, c_batch, self_dtype="bf16"):
        self.N = n_nodes
        self.IN_FEAT = in_feat
        self.HIDDEN = hidden
        self.N_CLASSES = n_classes
        self.NC = n_cores
        self.N_C = n_c                    # nodes per core (id // N_C)
        assert n_c * n_cores >= n_nodes
        self.BLK = blk                    # max dsts per block
        self.KCOL = kcol                  # columns per (block, stream)
        self.CAP = kcol * 128             # max edges per (block, stream)
        self.B_FIX = b_fix                # blocks per core (uniform)
        self.NQ = nq                      # source windows / gather streams
        self.SLOTS_C = b_fix * blk        # table slots per core
        assert self.SLOTS_C % 128 == 0
        self.NT = self.SLOTS_C // 128     # node tiles per core
        assert self.NT % 2 == 0
        self.TABLE_N = n_cores * self.SLOTS_C
        assert self.TABLE_N % nq == 0
        self.WIN = self.TABLE_N // nq     # table rows per source window
        assert self.WIN <= 32767          # int16 gather index range
        assert (n_cores % nq) == 0
        self.COLS_Q = b_fix * kcol        # gather columns per stream
        self.C_BATCH = c_batch            # columns per gather batch
        assert c_batch % kcol == 0 and self.COLS_Q % c_batch == 0
        self.N_BATCH = self.COLS_Q // c_batch
        self.BPB = c_batch // kcol        # blocks per batch
        # block pairs may straddle batches: the PSUM pair accumulator
        # persists across batch iterations and chunk logic keys on t
        self.SELF_DT = BF16 if self_dtype == "bf16" else F32
        self.NP_SELF = NP_BF16 if self_dtype == "bf16" else np.float32
        self.NCHUNK = 8                   # AllGather chunks per layer
        assert self.NCHUNK % nq == 0 and self.NT % self.NCHUNK == 0


CFG_FULL = Cfg(n_nodes=100000, in_feat=128, hidden=64, n_classes=16,
               n_cores=8, n_c=12544, blk=64, kcol=2, b_fix=208, nq=4,
               c_batch=26)


# ---------------------------------------------------------------------------
# Host-side preprocessing (sharding): all integer graph restructuring.
# ---------------------------------------------------------------------------

def preprocess(cfg, x, edge_index, W1, b1, W2, b2, Wl, bl):
    N, NC, N_C = cfg.N, cfg.NC, cfg.N_C
    src = np.asarray(edge_index[0]).astype(np.int64)
    dst = np.asarray(edge_index[1]).astype(np.int64)
    x = np.asarray(x, dtype=np.float32)

    deg = np.bincount(dst, minlength=N).astype(np.float32) + 1.0
    dinv = (1.0 / np.sqrt(deg)).astype(np.float32)

    # stream = stripe of the SOURCE node, chosen a priori and independent
    # of its parity class: stripe k nodes get packed into blocks
    # [56k, 56(k+1)) of their core, so table window q is the contiguous
    # rows [q*TABLE_N/NQ, ...) written by the q-th chunked AllGather.
    q_of = ((src // 2) % cfg.NQ).astype(np.int64)
    p_of = (src % 2).astype(np.int64)        # a-priori source parity class

    # per-(node, q, par) incoming edge counts
    degqp = np.bincount((dst * cfg.NQ + q_of) * 2 + p_of,
                        minlength=N * cfg.NQ * 2).reshape(N, cfg.NQ, 2)

    # --- per-core first-fit-decreasing packing of dsts into blocks ---
    # Constraints: per (block, q, par) <= 128 edges (one 128-row column);
    # node with id parity p gets a block position r with r % 2 == p, so
    # slot parity == id parity (known before packing any core).
    HBLK = cfg.BLK // 2
    B_STR = cfg.B_FIX // cfg.NQ          # blocks per stripe
    node_q = ((np.arange(NC * N_C) // 2) % cfg.NQ)
    slot_of = np.full(NC * N_C, -1, dtype=np.int64)
    node_of_slot = np.full(cfg.TABLE_N, -1, dtype=np.int64)
    for c in range(NC):
        lo, hi = c * N_C, min((c + 1) * N_C, N)
        if hi <= lo:
            continue
        for k in range(cfg.NQ):
            ids = lo + np.flatnonzero(node_q[lo:hi] == k)
            dq = degqp[ids].reshape(ids.size, cfg.NQ * 2)
            order = np.argsort(-dq.max(axis=1), kind="stable")
            accs = np.zeros((B_STR, cfg.NQ * 2), dtype=np.int64)
            cnts = np.zeros((B_STR, 2), dtype=np.int64)
            nopen = B_STR    # all blocks open: min-max-load spreads best
            for j in order:
                v = dq[j]
                g = int(ids[j])
                pj = g % 2
                fits = (cnts[:nopen, pj] < HBLK) & \
                       np.all(accs[:nopen] + v <= 128, axis=1)
                w = np.flatnonzero(fits)
                if w.size == 0:
                    assert nopen < B_STR, \
                        f"core {c} stripe {k}: packing exceeds {B_STR} blocks"
                    b = nopen
                    nopen += 1
                else:
                    # min-max-load: place where the worst cell stays lowest
                    b = int(w[np.argmin((accs[w] + v).max(axis=1))])
                r = 2 * cnts[b, pj] + pj
                s = c * cfg.SLOTS_C + (k * B_STR + b) * cfg.BLK + r
                slot_of[g] = s
                node_of_slot[s] = g
                accs[b] += v
                cnts[b, pj] += 1

    slot_of = slot_of[:N]

    # --- per-core edge streams ---
    e_core = dst // N_C
    s_slot = slot_of[src]
    d_slot_l = slot_of[dst] - e_core * cfg.SLOTS_C
    e_b = d_slot_l // cfg.BLK
    e_r = d_slot_l % cfg.BLK

    P_Q = cfg.B_FIX * cfg.CAP            # positions per stream
    idx_all = np.zeros((NC, cfg.NQ, P_Q), dtype=np.int16)

    e_par = (s_slot % 2).astype(np.int64)    # == src % 2 by construction
    order2 = np.lexsort((e_par, e_b, q_of, e_core))
    es_c, eq_c, eb_c = e_core[order2], q_of[order2], e_b[order2]
    ep_c = e_par[order2]
    grp = ((es_c * cfg.NQ + eq_c) * cfg.B_FIX + eb_c) * 2 + ep_c
    _, start_idx, cnt_grp = np.unique(grp, return_index=True,
                                      return_counts=True)
    rank = np.arange(grp.size) - np.repeat(start_idx, cnt_grp)
    assert rank.max(initial=0) < 128
    # column = 2*block + parity; position = column*128 + rank
    pos = eb_c * cfg.CAP + ep_c * 128 + rank
    s_sorted = s_slot[order2]
    # Table layout is chunk-major: chunk k (k = local_slot // CHR, CHR =
    # SLOTS_C/NCHUNK) holds every core's rows [c*CHR, (c+1)*CHR).
    # Window q = chunks [q*CPW, (q+1)*CPW) is still contiguous.
    STR_ROWS = cfg.SLOTS_C // cfg.NQ
    CHR = cfg.SLOTS_C // cfg.NCHUNK
    CPW = cfg.NCHUNK // cfg.NQ           # chunks per window
    l_sorted = s_sorted % cfg.SLOTS_C
    assert np.all(l_sorted // STR_ROWS == eq_c)
    k_l = l_sorted // CHR
    wrow = (k_l % CPW) * (cfg.NC * CHR) + \
        (s_sorted // cfg.SLOTS_C) * CHR + (l_sorted % CHR)
    # gather PAIR index (two table rows per 256B element)
    idx_val = (wrow // 2).astype(np.int16)
    idx_all[es_c, eq_c, pos] = idx_val

    # wrapped int16 layout: position i -> [i%16, i//16], replicated x8
    idx_w = idx_all.reshape(NC, cfg.NQ, -1, 16).transpose(0, 1, 3, 2)
    idx_dev = np.ascontiguousarray(np.tile(idx_w, (1, 1, 8, 1)))
    # one idx tensor per core, batch-major so each batch is ONE load:
    # [128, N_BATCH, NQ, C_BATCH*8]
    CB8 = cfg.C_BATCH * 8
    idx_cat = np.ascontiguousarray(
        idx_dev.reshape(NC, cfg.NQ, 128, cfg.N_BATCH, CB8)
        .transpose(0, 2, 3, 1, 4)
        .reshape(NC, 128, cfg.N_BATCH * cfg.NQ * CB8))

    # host-built one-hot selection matrices, bf16:
    # S[core, q, 128, col*BLK + d] = 1 iff edge at (partition, col) has
    # dst-row d within its block. Padding positions stay all-zero.
    S_np = np.zeros((NC, cfg.NQ, cfg.COLS_Q, 128, cfg.BLK), dtype=NP_BF16)
    S_np[es_c, eq_c, pos // 128, pos % 128, e_r[order2]] = 1.0
    S_dev = np.ascontiguousarray(
        S_np.transpose(0, 1, 3, 2, 4).reshape(
            NC, cfg.NQ, 128, cfg.COLS_Q * cfg.BLK))
    del S_np

    # --- per-slot node data ---
    valid = node_of_slot >= 0
    xe = np.zeros((cfg.TABLE_N, cfg.IN_FEAT), dtype=np.float32)
    xe[valid] = x[node_of_slot[valid]]
    dinv_s = np.zeros(cfg.TABLE_N, dtype=np.float32)
    dinv_s[valid] = dinv[node_of_slot[valid]]

    W1 = np.asarray(W1, np.float32)
    W2 = np.asarray(W2, np.float32).astype(NP_BF16)
    Wl = np.asarray(Wl, np.float32).astype(NP_BF16)
    b1 = np.asarray(b1, np.float32)
    b2 = np.asarray(b2, np.float32)
    bl = np.asarray(bl, np.float32)

    ident2 = np.concatenate([np.eye(cfg.HIDDEN), np.eye(cfg.HIDDEN)],
                            axis=0).astype(cfg.NP_SELF)

    in_maps = []
    for c in range(NC):
        sl = slice(c * cfg.SLOTS_C, (c + 1) * cfg.SLOTS_C)
        dv = dinv_s[sl]
        m = {
            "xT": np.ascontiguousarray(xe[sl].T),
            "w1": W1, "w2": W2, "wl": Wl,
            "b1c": b1.reshape(-1, 1), "b2c": b2.reshape(-1, 1),
            "blrep": np.tile(bl[None, :], (128, 1)),
            "dinvn": np.ascontiguousarray(dv.reshape(cfg.NT, 128).T),
            "dinvfm": np.tile(dv[None, :], (cfg.HIDDEN, 1)),
            "ident2": ident2,
        }
        m["idxall"] = idx_cat[c]
        for q in range(cfg.NQ):
            m[f"S{q}"] = S_dev[c, q]
        in_maps.append(m)

    return in_maps, node_of_slot


def assemble_output(cfg, results, node_of_slot):
    out = np.zeros((cfg.N, cfg.N_CLASSES), dtype=np.float32)
    for c, r in enumerate(results):
        lg = r["logits"].reshape(128, cfg.NT, cfg.N_CLASSES)
        sl = node_of_slot[c * cfg.SLOTS_C:(c + 1) * cfg.SLOTS_C]\
            .reshape(cfg.NT, 128)
        for t in range(cfg.NT):
            v = sl[t] >= 0
            out[sl[t][v]] = lg[v, t, :]
    return out


# ---------------------------------------------------------------------------
# Device program
# ---------------------------------------------------------------------------

def build_program(cfg):
    nc = bacc.Bacc("TRN2", target_bir_lowering=False, debug=False,
                   num_devices=cfg.NC, num_swdge_queues=4,
                   dynamic_dma_scratch_size=32768)
    H, NT = cfg.HIDDEN, cfg.NT

    xT_d = nc.dram_tensor("xT", [cfg.IN_FEAT, cfg.SLOTS_C], F32,
                          kind="ExternalInput")
    w1_d = nc.dram_tensor("w1", [cfg.IN_FEAT, H], F32, kind="ExternalInput")
    w2_d = nc.dram_tensor("w2", [H, H], BF16, kind="ExternalInput")
    wl_d = nc.dram_tensor("wl", [H, cfg.N_CLASSES], BF16,
                          kind="ExternalInput")
    b1c_d = nc.dram_tensor("b1c", [H, 1], F32, kind="ExternalInput")
    b2c_d = nc.dram_tensor("b2c", [H, 1], F32, kind="ExternalInput")
    blrep_d = nc.dram_tensor("blrep", [128, cfg.N_CLASSES], F32,
                             kind="ExternalInput")
    dinvn_d = nc.dram_tensor("dinvn", [128, NT], F32, kind="ExternalInput")
    dinvfm_d = nc.dram_tensor("dinvfm", [H, cfg.SLOTS_C], F32,
                              kind="ExternalInput")
    ident_d = nc.dram_tensor("ident2", [128, H], cfg.SELF_DT,
                             kind="ExternalInput")
    idx_d = nc.dram_tensor("idxall",
                           [128, cfg.N_BATCH * cfg.NQ * cfg.C_BATCH * 8],
                           I16, kind="ExternalInput")
    S_d = [nc.dram_tensor(f"S{q}", [128, cfg.COLS_Q * cfg.BLK], BF16,
                          kind="ExternalInput") for q in range(cfg.NQ)]
    logits_d = nc.dram_tensor("logits", [128, NT * cfg.N_CLASSES], F32,
                              kind="ExternalOutput")

    rg = [list(range(cfg.NC))]

    with tile.TileContext(nc) as tc:
        with tc.tile_pool(name="const", bufs=1) as cpool, \
             tc.tile_pool(name="dram", bufs=1, space="DRAM") as dpool, \
             tc.tile_pool(name="hp", bufs=3) as hpool:

            # hs chunk tiles: collective k fires as soon as its slice of
            # the local shard is written, overlapping the producing layer.
            NCHUNK = cfg.NCHUNK
            CH = cfg.SLOTS_C // NCHUNK
            TPC = NT // NCHUNK            # 128-row tiles per chunk
            hs1_c = [dpool.tile([CH, H], BF16, tag=f"hs1c{k}",
                                name=f"hs1c{k}") for k in range(NCHUNK)]
            hs2_c = [dpool.tile([CH, H], BF16, tag=f"hs2c{k}",
                                name=f"hs2c{k}") for k in range(NCHUNK)]
            tab1_t = dpool.tile([cfg.TABLE_N, H], BF16, tag="tab1",
                                name="tab1_t")
            tab2_t = dpool.tile([cfg.TABLE_N, H], BF16, tag="tab2",
                                name="tab2_t")
            # chunk k of the table = contiguous rows (stripe-major layout)
            CHT = cfg.TABLE_N // NCHUNK
            tab1_v = [tab1_t[k * CHT:(k + 1) * CHT, :]
                      for k in range(NCHUNK)]
            tab2_v = [tab2_t[k * CHT:(k + 1) * CHT, :]
                      for k in range(NCHUNK)]

            def cload(dram, shape, dt, tag):
                t = cpool.tile(shape, dt, tag=tag)
                nc.sync.dma_start(out=t[:], in_=dram[:, :])
                return t

            w1_s = cload(w1_d, [cfg.IN_FEAT, H], F32, "w1")
            w2_s = cload(w2_d, [H, H], BF16, "w2")
            wl_s = cload(wl_d, [H, cfg.N_CLASSES], BF16, "wl")
            b1c_s = cload(b1c_d, [H, 1], F32, "b1c")
            b2c_s = cload(b2c_d, [H, 1], F32, "b2c")
            blrep_s = cload(blrep_d, [128, cfg.N_CLASSES], F32, "blrep")
            dinvn_s = cload(dinvn_d, [128, NT], F32, "dinvn")
            ident_s = cload(ident_d, [128, H], cfg.SELF_DT, "ident")

            self1_s = cpool.tile([128, NT * H], cfg.SELF_DT, tag="self1")
            self2_s = cpool.tile([128, NT * H], cfg.SELF_DT, tag="self2")
            stageL_s = cpool.tile([128, NT * cfg.N_CLASSES], F32, tag="stgL")

            # staged hs rows: accumulate a whole chunk in SBUF, then ONE
            # DMA to DRAM per chunk (keeps waiting writes off the Sync
            # sequencer, whose in-order HWDGE ring would head-of-line
            # block the latency-critical idx loads)
            def stage_out_ap(hsc_k):
                return hsc_k[:, :].rearrange("(t p) h -> p t h", p=128)

            # ---- phase A: table1 = dinv * (x @ W1), plus self terms ----
            with tc.tile_pool(name="xp", bufs=1) as xpool, \
                 tc.tile_pool(name="stg", bufs=2) as stgpool, \
                 tc.tile_pool(name="pA", bufs=2, space="PSUM") as pA:
                xt_s = xpool.tile([cfg.IN_FEAT, cfg.SLOTS_C], F32, tag="xt")
                for k in range(NCHUNK):
                    nc.sync.dma_start(out=xt_s[:, k * CH:(k + 1) * CH],
                                      in_=xT_d[:, k * CH:(k + 1) * CH])
                stg = None
                for t in range(NT):
                    k, tk = t // TPC, t % TPC
                    if tk == 0:
                        stg = stgpool.tile([128, TPC, H], BF16, tag="stg1")
                    ps = pA.tile([128, H], F32, tag="a")
                    nc.tensor.matmul(out=ps[:],
                                     lhsT=xt_s[:, t * 128:(t + 1) * 128],
                                     rhs=w1_s[:], start=True, stop=True)
                    nc.vector.tensor_scalar_mul(out=stg[:, tk, :], in0=ps[:],
                                                scalar1=dinvn_s[:, t:t + 1])
                    # self-scale on Scalar: it is idle during phase A (no
                    # relu yet), and this halves the serial DVE chain that
                    # gates the chunk-collective fires -> earlier first batch
                    nc.scalar.activation(
                        out=self1_s[:, t * H:(t + 1) * H], in_=ps[:],
                        func=mybir.ActivationFunctionType.Copy,
                        scale=dinvn_s[:, t:t + 1])
                    if tk == TPC - 1:
                        nc.scalar.dma_start(out=stage_out_ap(hs1_c[k]),
                                            in_=stg[:])
                        nc.gpsimd.collective_compute(
                            "AllGather", mybir.AluOpType.bypass,
                            replica_groups=rg, ins=[hs1_c[k][:, :]],
                            outs=[tab1_v[k]])

            # ---- phases B (layer1 -> table2) and C (layer2 -> logits) ----
            with tc.tile_pool(name="sp", bufs=3) as spool, \
                 tc.tile_pool(name="stg2", bufs=2) as stg2pool, \
                 tc.tile_pool(name="pp", bufs=2, space="PSUM") as pp:

                def conv_layer(layer):
                    tab_t = tab1_t if layer == 1 else tab2_t
                    self_s = self1_s if layer == 1 else self2_s
                    bc_s = b1c_s if layer == 1 else b2c_s
                    # paired-row view of the table: one 256B gather element
                    # covers two consecutive bf16 rows (slots 2w, 2w+1)
                    tabp = tab_t[:].rearrange("(n two) h -> n (two h)", two=2)
                    pair = {}
                    fired = [False] * NCHUNK

                    def fire_tab2(k):
                        nc.gpsimd.collective_compute(
                            "AllGather", mybir.AluOpType.bypass,
                            replica_groups=rg, ins=[hs2_c[k][:, :]],
                            outs=[tab2_v[k]])
                        fired[k] = True

                    # layer 1 runs batches 4..15 first so stream 0's table
                    # chunks (written by batches 0..3) land last — its
                    # layer-2 gather is the one issued last per batch, so
                    # the remaining collective latency hides behind the
                    # other streams' descriptor generation.
                    border = (list(range(4, cfg.N_BATCH)) + list(range(4))
                              if layer == 1 else list(range(cfg.N_BATCH)))
                    BPC = cfg.N_BATCH // NCHUNK
                    done_pos = [max(border.index(b)
                                    for b in range(k * BPC, (k + 1) * BPC))
                                for k in range(NCHUNK)]

                    CB8 = cfg.C_BATCH * 8
                    for j in range(cfg.N_BATCH):
                        i = border[j]
                        msgs = [None] * (2 * cfg.NQ)
                        Ss = [None] * cfg.NQ
                        idx_t = spool.tile([128, cfg.NQ * CB8], I16,
                                           tag="idx")
                        nc.sync.dma_start(
                            out=idx_t[:],
                            in_=idx_d[:, i * cfg.NQ * CB8:
                                      (i + 1) * cfg.NQ * CB8])
                        CH2 = cfg.C_BATCH // 2
                        for h in range(2):
                            # two half-gathers per stream: staggers the
                            # doorbells so SDMA engines always have fresh
                            # packets, and overlaps each queue's second
                            # generation with its first drain
                            for q in (list(range(1, cfg.NQ)) + [0]):
                                msg_t = spool.tile([128, CH2, 2 * H],
                                                   BF16, tag=f"msg{q}_{h}")
                                nc.gpsimd.dma_gather(
                                    out_ap=msg_t[:],
                                    in_ap=tabp[q * cfg.WIN // 2:
                                               (q + 1) * cfg.WIN // 2, :],
                                    idxs_ap=idx_t[:, q * CB8 + h * CB8 // 2:
                                                  q * CB8 + (h + 1) *
                                                  CB8 // 2],
                                    num_idxs=CH2 * 128,
                                    num_idxs_reg=CH2 * 128,
                                    elem_size=2 * H, queue_num=q,
                                    single_packet=False)
                                msgs[q * 2 + h] = \
                                    msg_t[:].rearrange("p c f -> p (c f)")
                        if layer == 1:
                            # fire chunk collectives two processed batches
                            # after their last row landed, AFTER this batch's
                            # gathers, so the (in-order) Pool trigger never
                            # blocks them
                            for k in range(NCHUNK):
                                if not fired[k] and j >= done_pos[k] + 3:
                                    fire_tab2(k)
                        for q in range(cfg.NQ):
                            S_t = spool.tile([128, cfg.C_BATCH * cfg.BLK],
                                             BF16, tag=f"S{q}")
                            # ACT-ring HWDGE: keeps the big S loads off the
                            # Sync sequencer, which handles everything else
                            nc.scalar.dma_start(
                                out=S_t[:],
                                in_=S_d[q][:, i * cfg.C_BATCH * cfg.BLK:
                                           (i + 1) * cfg.C_BATCH * cfg.BLK])
                            Ss[q] = S_t[:]

                        dfm_t = spool.tile([H, cfg.BPB * cfg.BLK], F32,
                                           tag="dfm")
                        nc.sync.dma_start(
                            out=dfm_t[:],
                            in_=dinvfm_d[:, i * cfg.BPB * cfg.BLK:
                                         (i + 1) * cfg.BPB * cfg.BLK])

                        for bb in range(cfg.BPB):
                            b = i * cfg.BPB + bb
                            half = (b % 2) * H
                            t = b // 2
                            pfm = pp.tile([H, cfg.BLK], F32, tag="fm")
                            nc.tensor.matmul(
                                out=pfm[:],
                                lhsT=self_s[half:half + H,
                                            t * H:(t + 1) * H],
                                rhs=ident_s[half:half + H, :],
                                start=True, stop=False)
                            for q in range(cfg.NQ):
                                for k in range(cfg.KCOL):
                                    lc = bb * cfg.KCOL + k
                                    last = (q == cfg.NQ - 1 and
                                            k == cfg.KCOL - 1)
                                    mh, lcl = lc // CH2, lc % CH2
                                    # column parity k selects the half of
                                    # the gathered pair element
                                    nc.tensor.matmul(
                                        out=pfm[:],
                                        lhsT=msgs[q * 2 + mh][
                                            :, lcl * 2 * H + k * H:
                                            lcl * 2 * H + (k + 1) * H],
                                        rhs=Ss[q][:, lc * cfg.BLK:
                                                  (lc + 1) * cfg.BLK],
                                        start=False, stop=last)
                            h_t = hpool.tile([H, cfg.BLK], F32, tag="h")
                            nc.vector.tensor_tensor(
                                out=h_t[:], in0=pfm[:],
                                in1=dfm_t[:, bb * cfg.BLK:(bb + 1) * cfg.BLK],
                                op=mybir.AluOpType.mult)
                            hr_t = hpool.tile([H, cfg.BLK], BF16, tag="hr")
                            nc.scalar.activation(
                                out=hr_t[:], in_=h_t[:],
                                func=mybir.ActivationFunctionType.Relu,
                                bias=bc_s[:])
                            if layer == 1:
                                if b % 2 == 0:
                                    pair["p2"] = pp.tile([128, H], F32, name="p2",
                                                         tag="pair")
                                p2 = pair["p2"]
                                nc.tensor.matmul(
                                    out=p2[half:half + H, :], lhsT=hr_t[:],
                                    rhs=w2_s[:], start=True, stop=True,
                                    tile_position=(0, half))
                                if b % 2 == 1:
                                    k, tk = t // TPC, t % TPC
                                    if tk == 0:
                                        pair["stg"] = stg2pool.tile(
                                            [128, TPC, H], BF16, name="stg2",
                                            tag="stg2")
                                    nc.vector.tensor_scalar_mul(
                                        out=pair["stg"][:, tk, :], in0=p2[:],
                                        scalar1=dinvn_s[:, t:t + 1])
                                    nc.vector.tensor_scalar_mul(
                                        out=self2_s[:, t * H:(t + 1) * H],
                                        in0=p2[:],
                                        scalar1=dinvn_s[:, t:t + 1])
                                    if tk == TPC - 1:
                                        nc.scalar.dma_start(
                                            out=stage_out_ap(hs2_c[k]),
                                            in_=pair["stg"][:])
                            else:
                                if b % 2 == 0:
                                    pair["pl"] = pp.tile([128, cfg.N_CLASSES],
                                                         F32, name="pl", tag="pl")
                                pl = pair["pl"]
                                nc.tensor.matmul(
                                    out=pl[half:half + H, :], lhsT=hr_t[:],
                                    rhs=wl_s[:], start=True, stop=True,
                                    tile_position=(0, half))
                                if b % 2 == 1:
                                    nCL = cfg.N_CLASSES
                                    nc.vector.tensor_tensor(
                                        out=stageL_s[:, t * nCL:(t + 1) * nCL],
                                        in0=pl[:], in1=blrep_s[:],
                                        op=mybir.AluOpType.add)

                    if layer == 1:
                        for k in range(NCHUNK):
                            if not fired[k]:
                                fire_tab2(k)

                conv_layer(1)
                conv_layer(2)

            nc.sync.dma_start(out=logits_d[:, :], in_=stageL_s[:])

    nc.compile()
    return nc


_PROGRAM_CACHE = {}


def get_program(cfg):
    key = id(cfg)
    if key not in _PROGRAM_CACHE:
        _PROGRAM_CACHE[key] = build_program(cfg)
    return _PROGRAM_CACHE[key]


def run(cfg, inputs, trace=False):
    in_maps, node_of_slot = preprocess(cfg, **inputs)
    nc = get_program(cfg)
    res = bass_utils.run_bass_kernel_spmd(
        nc, in_maps, core_ids=list(range(cfg.NC)), trace=trace)
    out = assemble_output(cfg, res.results, node_of_slot)
    return out, res


def kernel(**inputs) -> np.ndarray:
    out, _ = run(CFG_FULL, inputs)
    return out



# revision 69
# speedup vs baseline: 1.2865x; 1.0064x over previous
"""2-layer GCN (GCNConv+relu x2, linear head) on 8 Trainium2 NeuronCores.

Strategy (graph/data parallel, per sharding hint):
  - Nodes sharded across 8 cores by id; edges partitioned by destination.
  - Per core, destination nodes are bin-packed into B_FIX blocks of <=BLK
    dsts such that each (block, source-window) holds <= KCOL*128 edges.
    This gives an SPMD-uniform program; only tensor data varies per core.
  - Per layer: local matmul (x@W scaled by dinv) -> AllGather into a
    full node-major bf16 table in DRAM -> dma_gather one 256B element
    per edge = a PAIR of bf16 rows (slots 2w, 2w+1); parity-split
    selection matrices S_even/S_odd (is_equal on DVE, bf16) route the
    correct half; PE bf16 matmuls accumulate feature-major conv output
    in PSUM; self-loop terms enter via an identity-matmul transpose.
    Post: scale by dinv, +bias, relu, next-layer matmul (bf16).
  - The 4 source-window gathers go to 4 SWDGE queues: each queue's
    descriptor generation runs on its own GpSimd Q7 core pair, so the
    4 gathers of a batch overlap (queue 0 issued last since its pair
    is the one the engine timeline blocks on).
"""

import numpy as np

import concourse.bass as bass
import concourse.mybir as mybir
import concourse.tile as tile
from concourse import bacc
from concourse import bass_utils

import ml_dtypes

F32 = mybir.dt.float32
BF16 = mybir.dt.bfloat16
I16 = mybir.dt.int16
NP_BF16 = ml_dtypes.bfloat16


class Cfg:
    def __init__(self, n_nodes, in_feat, hidden, n_classes, n_cores, n_c,
                 blk, kcol, b_fix, nq, c_batch, self_dtype="bf16"):
        self.N = n_nodes
        self.IN_FEAT = in_feat
        self.HIDDEN = hidden
        self.N_CLASSES = n_classes
        self.NC = n_cores
        self.N_C = n_c                    # nodes per core (id // N_C)
        assert n_c * n_cores >= n_nodes
        self.BLK = blk                    # max dsts per block
        self.KCOL = kcol                  # columns per (block, stream)
        self.CAP = kcol * 128             # max edges per (block, stream)
        self.B_FIX = b_fix                # blocks per core (uniform)
        self.NQ = nq                      # source windows / gather streams
        self.SLOTS_C = b_fix * blk        # table slots per core
        assert self.SLOTS_C % 128 == 0
        self.NT = self.SLOTS_C // 128     # node tiles per core
        assert self.NT % 2 == 0
        self.TABLE_N = n_cores * self.SLOTS_C
        assert self.TABLE_N % nq == 0
        self.WIN = self.TABLE_N // nq     # table rows per source window
        assert self.WIN <= 32767          # int16 gather index range
        assert (n_cores % nq) == 0
        self.COLS_Q = b_fix * kcol        # gather columns per stream
        self.C_BATCH = c_batch            # columns per gather batch
        assert c_batch % kcol == 0 and self.COLS_Q % c_batch == 0
        self.N_BATCH = self.COLS_Q // c_batch
        self.BPB = c_batch // kcol        # blocks per batch
        # block pairs may straddle batches: the PSUM pair accumulator
        # persists across batch iterations and chunk logic keys on t
        self.SELF_DT = BF16 if self_dtype == "bf16" else F32
        self.NP_SELF = NP_BF16 if self_dtype == "bf16" else np.float32
        self.NCHUNK = 8                   # AllGather chunks per layer
        assert self.NCHUNK % nq == 0 and self.NT % self.NCHUNK == 0


CFG_FULL = Cfg(n_nodes=100000, in_feat=128, hidden=64, n_classes=16,
               n_cores=8, n_c=12544, blk=64, kcol=2, b_fix=208, nq=4,
               c_batch=26)


# ---------------------------------------------------------------------------
# Host-side preprocessing (sharding): all integer graph restructuring.
# ---------------------------------------------------------------------------

def preprocess(cfg, x, edge_index, W1, b1, W2, b2, Wl, bl):
    N, NC, N_C = cfg.N, cfg.NC, cfg.N_C
    src = np.asarray(edge_index[0]).astype(np.int64)
    dst = np.asarray(edge_index[1]).astype(np.int64)
    x = np.asarray(x, dtype=np.float32)

    deg = np.bincount(dst, minlength=N).astype(np.float32) + 1.0
    dinv = (1.0 / np.sqrt(deg)).astype(np.float32)

    # stream = stripe of the SOURCE node, chosen a priori and independent
    # of its parity class: stripe k nodes get packed into blocks
    # [56k, 56(k+1)) of their core, so table window q is the contiguous
    # rows [q*TABLE_N/NQ, ...) written by the q-th chunked AllGather.
    q_of = ((src // 2) % cfg.NQ).astype(np.int64)
    p_of = (src % 2).astype(np.int64)        # a-priori source parity class

    # per-(node, q, par) incoming edge counts
    degqp = np.bincount((dst * cfg.NQ + q_of) * 2 + p_of,
                        minlength=N * cfg.NQ * 2).reshape(N, cfg.NQ, 2)

    # --- per-core first-fit-decreasing packing of dsts into blocks ---
    # Constraints: per (block, q, par) <= 128 edges (one 128-row column);
    # node with id parity p gets a block position r with r % 2 == p, so
    # slot parity == id parity (known before packing any core).
    HBLK = cfg.BLK // 2
    B_STR = cfg.B_FIX // cfg.NQ          # blocks per stripe
    node_q = ((np.arange(NC * N_C) // 2) % cfg.NQ)
    slot_of = np.full(NC * N_C, -1, dtype=np.int64)
    node_of_slot = np.full(cfg.TABLE_N, -1, dtype=np.int64)
    for c in range(NC):
        lo, hi = c * N_C, min((c + 1) * N_C, N)
        if hi <= lo:
            continue
        for k in range(cfg.NQ):
            ids = lo + np.flatnonzero(node_q[lo:hi] == k)
            dq = degqp[ids].reshape(ids.size, cfg.NQ * 2)
            order = np.argsort(-dq.max(axis=1), kind="stable")
            accs = np.zeros((B_STR, cfg.NQ * 2), dtype=np.int64)
            cnts = np.zeros((B_STR, 2), dtype=np.int64)
            nopen = B_STR    # all blocks open: min-max-load spreads best
            for j in order:
                v = dq[j]
                g = int(ids[j])
                pj = g % 2
                fits = (cnts[:nopen, pj] < HBLK) & \
                       np.all(accs[:nopen] + v <= 128, axis=1)
                w = np.flatnonzero(fits)
                if w.size == 0:
                    assert nopen < B_STR, \
                        f"core {c} stripe {k}: packing exceeds {B_STR} blocks"
                    b = nopen
                    nopen += 1
                else:
                    # min-max-load: place where the worst cell stays lowest
                    b = int(w[np.argmin((accs[w] + v).max(axis=1))])
                r = 2 * cnts[b, pj] + pj
                s = c * cfg.SLOTS_C + (k * B_STR + b) * cfg.BLK + r
                slot_of[g] = s
                node_of_slot[s] = g
                accs[b] += v
                cnts[b, pj] += 1

    slot_of = slot_of[:N]

    # --- per-core edge streams ---
    e_core = dst // N_C
    s_slot = slot_of[src]
    d_slot_l = slot_of[dst] - e_core * cfg.SLOTS_C
    e_b = d_slot_l // cfg.BLK
    e_r = d_slot_l % cfg.BLK

    P_Q = cfg.B_FIX * cfg.CAP            # positions per stream
    idx_all = np.zeros((NC, cfg.NQ, P_Q), dtype=np.int16)

    e_par = (s_slot % 2).astype(np.int64)    # == src % 2 by construction
    order2 = np.lexsort((e_par, e_b, q_of, e_core))
    es_c, eq_c, eb_c = e_core[order2], q_of[order2], e_b[order2]
    ep_c = e_par[order2]
    grp = ((es_c * cfg.NQ + eq_c) * cfg.B_FIX + eb_c) * 2 + ep_c
    _, start_idx, cnt_grp = np.unique(grp, return_index=True,
                                      return_counts=True)
    rank = np.arange(grp.size) - np.repeat(start_idx, cnt_grp)
    assert rank.max(initial=0) < 128
    # column = 2*block + parity; position = column*128 + rank
    pos = eb_c * cfg.CAP + ep_c * 128 + rank
    s_sorted = s_slot[order2]
    # Table layout is chunk-major: chunk k (k = local_slot // CHR, CHR =
    # SLOTS_C/NCHUNK) holds every core's rows [c*CHR, (c+1)*CHR).
    # Window q = chunks [q*CPW, (q+1)*CPW) is still contiguous.
    STR_ROWS = cfg.SLOTS_C // cfg.NQ
    CHR = cfg.SLOTS_C // cfg.NCHUNK
    CPW = cfg.NCHUNK // cfg.NQ           # chunks per window
    l_sorted = s_sorted % cfg.SLOTS_C
    assert np.all(l_sorted // STR_ROWS == eq_c)
    k_l = l_sorted // CHR
    wrow = (k_l % CPW) * (cfg.NC * CHR) + \
        (s_sorted // cfg.SLOTS_C) * CHR + (l_sorted % CHR)
    # gather PAIR index (two table rows per 256B element)
    idx_val = (wrow // 2).astype(np.int16)
    idx_all[es_c, eq_c, pos] = idx_val

    # wrapped int16 layout: position i -> [i%16, i//16], replicated x8
    idx_w = idx_all.reshape(NC, cfg.NQ, -1, 16).transpose(0, 1, 3, 2)
    idx_dev = np.ascontiguousarray(np.tile(idx_w, (1, 1, 8, 1)))
    # one idx tensor per core, batch-major so each batch is ONE load:
    # [128, N_BATCH, NQ, C_BATCH*8]
    CB8 = cfg.C_BATCH * 8
    idx_cat = np.ascontiguousarray(
        idx_dev.reshape(NC, cfg.NQ, 128, cfg.N_BATCH, CB8)
        .transpose(0, 2, 3, 1, 4)
        .reshape(NC, 128, cfg.N_BATCH * cfg.NQ * CB8))

    # host-built one-hot selection matrices, bf16:
    # S[core, q, 128, col*BLK + d] = 1 iff edge at (partition, col) has
    # dst-row d within its block. Padding positions stay all-zero.
    S_np = np.zeros((NC, cfg.NQ, cfg.COLS_Q, 128, cfg.BLK), dtype=NP_BF16)
    S_np[es_c, eq_c, pos // 128, pos % 128, e_r[order2]] = 1.0
    S_dev = np.ascontiguousarray(
        S_np.transpose(0, 1, 3, 2, 4).reshape(
            NC, cfg.NQ, 128, cfg.COLS_Q * cfg.BLK))
    del S_np

    # --- per-slot node data ---
    valid = node_of_slot >= 0
    xe = np.zeros((cfg.TABLE_N, cfg.IN_FEAT), dtype=np.float32)
    xe[valid] = x[node_of_slot[valid]]
    dinv_s = np.zeros(cfg.TABLE_N, dtype=np.float32)
    dinv_s[valid] = dinv[node_of_slot[valid]]

    W1 = np.asarray(W1, np.float32)
    W2 = np.asarray(W2, np.float32).astype(NP_BF16)
    Wl = np.asarray(Wl, np.float32).astype(NP_BF16)
    b1 = np.asarray(b1, np.float32)
    b2 = np.asarray(b2, np.float32)
    bl = np.asarray(bl, np.float32)

    ident2 = np.concatenate([np.eye(cfg.HIDDEN), np.eye(cfg.HIDDEN)],
                            axis=0).astype(cfg.NP_SELF)

    in_maps = []
    for c in range(NC):
        sl = slice(c * cfg.SLOTS_C, (c + 1) * cfg.SLOTS_C)
        dv = dinv_s[sl]
        m = {
            "xT": np.ascontiguousarray(xe[sl].T),
            "w1": W1, "w2": W2, "wl": Wl,
            "b1c": b1.reshape(-1, 1), "b2c": b2.reshape(-1, 1),
            "blrep": np.tile(bl[None, :], (128, 1)),
            "dinvn": np.ascontiguousarray(dv.reshape(cfg.NT, 128).T),
            "dinvfm": np.tile(dv[None, :], (cfg.HIDDEN, 1)),
            "ident2": ident2,
        }
        m["idxall"] = idx_cat[c]
        for q in range(cfg.NQ):
            m[f"S{q}"] = S_dev[c, q]
        in_maps.append(m)

    return in_maps, node_of_slot


def assemble_output(cfg, results, node_of_slot):
    out = np.zeros((cfg.N, cfg.N_CLASSES), dtype=np.float32)
    for c, r in enumerate(results):
        lg = r["logits"].reshape(128, cfg.NT, cfg.N_CLASSES)
        sl = node_of_slot[c * cfg.SLOTS_C:(c + 1) * cfg.SLOTS_C]\
            .reshape(cfg.NT, 128)
        for t in range(cfg.NT):
            v = sl[t] >= 0
            out[sl[t][v]] = lg[v, t, :]
    return out


# ---------------------------------------------------------------------------
# Device program
# ---------------------------------------------------------------------------

def build_program(cfg):
    nc = bacc.Bacc("TRN2", target_bir_lowering=False, debug=False,
                   num_devices=cfg.NC, num_swdge_queues=4,
                   dynamic_dma_scratch_size=32768)
    H, NT = cfg.HIDDEN, cfg.NT

    xT_d = nc.dram_tensor("xT", [cfg.IN_FEAT, cfg.SLOTS_C], F32,
                          kind="ExternalInput")
    w1_d = nc.dram_tensor("w1", [cfg.IN_FEAT, H], F32, kind="ExternalInput")
    w2_d = nc.dram_tensor("w2", [H, H], BF16, kind="ExternalInput")
    wl_d = nc.dram_tensor("wl", [H, cfg.N_CLASSES], BF16,
                          kind="ExternalInput")
    b1c_d = nc.dram_tensor("b1c", [H, 1], F32, kind="ExternalInput")
    b2c_d = nc.dram_tensor("b2c", [H, 1], F32, kind="ExternalInput")
    blrep_d = nc.dram_tensor("blrep", [128, cfg.N_CLASSES], F32,
                             kind="ExternalInput")
    dinvn_d = nc.dram_tensor("dinvn", [128, NT], F32, kind="ExternalInput")
    dinvfm_d = nc.dram_tensor("dinvfm", [H, cfg.SLOTS_C], F32,
                              kind="ExternalInput")
    ident_d = nc.dram_tensor("ident2", [128, H], cfg.SELF_DT,
                             kind="ExternalInput")
    idx_d = nc.dram_tensor("idxall",
                           [128, cfg.N_BATCH * cfg.NQ * cfg.C_BATCH * 8],
                           I16, kind="ExternalInput")
    S_d = [nc.dram_tensor(f"S{q}", [128, cfg.COLS_Q * cfg.BLK], BF16,
                          kind="ExternalInput") for q in range(cfg.NQ)]
    logits_d = nc.dram_tensor("logits", [128, NT * cfg.N_CLASSES], F32,
                              kind="ExternalOutput")

    rg = [list(range(cfg.NC))]

    with tile.TileContext(nc) as tc:
        with tc.tile_pool(name="const", bufs=1) as cpool, \
             tc.tile_pool(name="dram", bufs=1, space="DRAM") as dpool, \
             tc.tile_pool(name="hp", bufs=3) as hpool:

            # hs chunk tiles: collective k fires as soon as its slice of
            # the local shard is written, overlapping the producing layer.
            NCHUNK = cfg.NCHUNK
            CH = cfg.SLOTS_C // NCHUNK
            TPC = NT // NCHUNK            # 128-row tiles per chunk
            hs1_c = [dpool.tile([CH, H], BF16, tag=f"hs1c{k}",
                                name=f"hs1c{k}") for k in range(NCHUNK)]
            hs2_c = [dpool.tile([CH, H], BF16, tag=f"hs2c{k}",
                                name=f"hs2c{k}") for k in range(NCHUNK)]
            tab1_t = dpool.tile([cfg.TABLE_N, H], BF16, tag="tab1",
                                name="tab1_t")
            tab2_t = dpool.tile([cfg.TABLE_N, H], BF16, tag="tab2",
                                name="tab2_t")
            # chunk k of the table = contiguous rows (stripe-major layout)
            CHT = cfg.TABLE_N // NCHUNK
            tab1_v = [tab1_t[k * CHT:(k + 1) * CHT, :]
                      for k in range(NCHUNK)]
            tab2_v = [tab2_t[k * CHT:(k + 1) * CHT, :]
                      for k in range(NCHUNK)]

            def cload(dram, shape, dt, tag):
                t = cpool.tile(shape, dt, tag=tag)
                nc.sync.dma_start(out=t[:], in_=dram[:, :])
                return t

            w1_s = cload(w1_d, [cfg.IN_FEAT, H], F32, "w1")
            w2_s = cload(w2_d, [H, H], BF16, "w2")
            wl_s = cload(wl_d, [H, cfg.N_CLASSES], BF16, "wl")
            b1c_s = cload(b1c_d, [H, 1], F32, "b1c")
            b2c_s = cload(b2c_d, [H, 1], F32, "b2c")
            blrep_s = cload(blrep_d, [128, cfg.N_CLASSES], F32, "blrep")
            dinvn_s = cload(dinvn_d, [128, NT], F32, "dinvn")
            ident_s = cload(ident_d, [128, H], cfg.SELF_DT, "ident")

            self1_s = cpool.tile([128, NT * H], cfg.SELF_DT, tag="self1")
            self2_s = cpool.tile([128, NT * H], cfg.SELF_DT, tag="self2")
            stageL_s = cpool.tile([128, NT * cfg.N_CLASSES], F32, tag="stgL")

            # staged hs rows: accumulate a whole chunk in SBUF, then ONE
            # DMA to DRAM per chunk (keeps waiting writes off the Sync
            # sequencer, whose in-order HWDGE ring would head-of-line
            # block the latency-critical idx loads)
            def stage_out_ap(hsc_k):
                return hsc_k[:, :].rearrange("(t p) h -> p t h", p=128)

            # ---- phase A: table1 = dinv * (x @ W1), plus self terms ----
            with tc.tile_pool(name="xp", bufs=1) as xpool, \
                 tc.tile_pool(name="stg", bufs=2) as stgpool, \
                 tc.tile_pool(name="pA", bufs=2, space="PSUM") as pA:
                xt_s = xpool.tile([cfg.IN_FEAT, cfg.SLOTS_C], F32, tag="xt")
                for k in range(NCHUNK):
                    nc.sync.dma_start(out=xt_s[:, k * CH:(k + 1) * CH],
                                      in_=xT_d[:, k * CH:(k + 1) * CH])
                stg = None
                for t in range(NT):
                    k, tk = t // TPC, t % TPC
                    if tk == 0:
                        stg = stgpool.tile([128, TPC, H], BF16, tag="stg1")
                    ps = pA.tile([128, H], F32, tag="a")
                    nc.tensor.matmul(out=ps[:],
                                     lhsT=xt_s[:, t * 128:(t + 1) * 128],
                                     rhs=w1_s[:], start=True, stop=True)
                    nc.vector.tensor_scalar_mul(out=stg[:, tk, :], in0=ps[:],
                                                scalar1=dinvn_s[:, t:t + 1])
                    # self-scale on Scalar: it is idle during phase A (no
                    # relu yet), and this halves the serial DVE chain that
                    # gates the chunk-collective fires -> earlier first batch
                    nc.scalar.activation(
                        out=self1_s[:, t * H:(t + 1) * H], in_=ps[:],
                        func=mybir.ActivationFunctionType.Copy,
                        scale=dinvn_s[:, t:t + 1])
                    if tk == TPC - 1:
                        nc.scalar.dma_start(out=stage_out_ap(hs1_c[k]),
                                            in_=stg[:])
                        nc.gpsimd.collective_compute(
                            "AllGather", mybir.AluOpType.bypass,
                            replica_groups=rg, ins=[hs1_c[k][:, :]],
                            outs=[tab1_v[k]])

            # ---- phases B (layer1 -> table2) and C (layer2 -> logits) ----
            with tc.tile_pool(name="sp", bufs=3) as spool, \
                 tc.tile_pool(name="stg2", bufs=2) as stg2pool, \
                 tc.tile_pool(name="pp", bufs=2, space="PSUM") as pp:

                def conv_layer(layer):
                    tab_t = tab1_t if layer == 1 else tab2_t
                    self_s = self1_s if layer == 1 else self2_s
                    bc_s = b1c_s if layer == 1 else b2c_s
                    # paired-row view of the table: one 256B gather element
                    # covers two consecutive bf16 rows (slots 2w, 2w+1)
                    tabp = tab_t[:].rearrange("(n two) h -> n (two h)", two=2)
                    pair = {}
                    fired = [False] * NCHUNK

                    def fire_tab2(k):
                        nc.gpsimd.collective_compute(
                            "AllGather", mybir.AluOpType.bypass,
                            replica_groups=rg, ins=[hs2_c[k][:, :]],
                            outs=[tab2_v[k]])
                        fired[k] = True

                    # layer 1 runs batches 4..15 first so stream 0's table
                    # chunks (written by batches 0..3) land last — its
                    # layer-2 gather is the one issued last per batch, so
                    # the remaining collective latency hides behind the
                    # other streams' descriptor generation.
                    border = (list(range(4, cfg.N_BATCH)) + list(range(4))
                              if layer == 1 else list(range(cfg.N_BATCH)))
                    BPC = cfg.N_BATCH // NCHUNK
                    done_pos = [max(border.index(b)
                                    for b in range(k * BPC, (k + 1) * BPC))
                                for k in range(NCHUNK)]

                    CB8 = cfg.C_BATCH * 8
                    for j in range(cfg.N_BATCH):
                        i = border[j]
                        msgs = [None] * (2 * cfg.NQ)
                        Ss = [None] * cfg.NQ
                        idx_t = spool.tile([128, cfg.NQ * CB8], I16,
                                           tag="idx")
                        nc.sync.dma_start(
                            out=idx_t[:],
                            in_=idx_d[:, i * cfg.NQ * CB8:
                                      (i + 1) * cfg.NQ * CB8])
                        CH2 = cfg.C_BATCH // 2
                        for h in range(2):
                            # two half-gathers per stream: staggers the
                            # doorbells so SDMA engines always have fresh
                            # packets, and overlaps each queue's second
                            # generation with its first drain
                            for q in (list(range(1, cfg.NQ)) + [0]):
                                msg_t = spool.tile([128, CH2, 2 * H],
                                                   BF16, tag=f"msg{q}_{h}")
                                nc.gpsimd.dma_gather(
                                    out_ap=msg_t[:],
                                    in_ap=tabp[q * cfg.WIN // 2:
                                               (q + 1) * cfg.WIN // 2, :],
                                    idxs_ap=idx_t[:, q * CB8 + h * CB8 // 2:
                                                  q * CB8 + (h + 1) *
                                                  CB8 // 2],
                                    num_idxs=CH2 * 128,
                                    num_idxs_reg=CH2 * 128,
                                    elem_size=2 * H, queue_num=q,
                                    single_packet=False)
                                msgs[q * 2 + h] = \
                                    msg_t[:].rearrange("p c f -> p (c f)")
                        if layer == 1:
                            # fire chunk collectives two processed batches
                            # after their last row landed, AFTER this batch's
                            # gathers, so the (in-order) Pool trigger never
                            # blocks them
                            for k in range(NCHUNK):
                                if not fired[k] and j >= done_pos[k] + 3:
                                    fire_tab2(k)
                        for q in range(cfg.NQ):
                            S_t = spool.tile([128, cfg.C_BATCH * cfg.BLK],
                                             BF16, tag=f"S{q}")
                            # ACT-ring HWDGE: keeps the big S loads off the
                            # Sync sequencer, which handles everything else
                            nc.scalar.dma_start(
                                out=S_t[:],
                                in_=S_d[q][:, i * cfg.C_BATCH * cfg.BLK:
                                           (i + 1) * cfg.C_BATCH * cfg.BLK])
                            Ss[q] = S_t[:]

                        dfm_t = spool.tile([H, cfg.BPB * cfg.BLK], F32,
                                           tag="dfm")
                        nc.sync.dma_start(
                            out=dfm_t[:],
                            in_=dinvfm_d[:, i * cfg.BPB * cfg.BLK:
                                         (i + 1) * cfg.BPB * cfg.BLK])

                        for bb in range(cfg.BPB):
                            b = i * cfg.BPB + bb
                            half = (b % 2) * H
                            t = b // 2
                            pfm = pp.tile([H, cfg.BLK], F32, tag="fm")
                            nc.tensor.matmul(
                                out=pfm[:],
                                lhsT=self_s[half:half + H,
                                            t * H:(t + 1) * H],
                                rhs=ident_s[half:half + H, :],
                                start=True, stop=False)
                            for q in range(cfg.NQ):
                                for k in range(cfg.KCOL):
                                    lc = bb * cfg.KCOL + k
                                    last = (q == cfg.NQ - 1 and
                                            k == cfg.KCOL - 1)
                                    mh, lcl = lc // CH2, lc % CH2
                                    # column parity k selects the half of
                                    # the gathered pair element
                                    nc.tensor.matmul(
                                        out=pfm[:],
                                        lhsT=msgs[q * 2 + mh][
                                            :, lcl * 2 * H + k * H:
                                            lcl * 2 * H + (k + 1) * H],
                                        rhs=Ss[q][:, lc * cfg.BLK:
                                                  (lc + 1) * cfg.BLK],
                                        start=False, stop=last)
                            h_t = hpool.tile([H, cfg.BLK], F32, tag="h")
                            nc.vector.tensor_tensor(
                                out=h_t[:], in0=pfm[:],
                                in1=dfm_t[:, bb * cfg.BLK:(bb + 1) * cfg.BLK],
                                op=mybir.AluOpType.mult)
                            hr_t = hpool.tile([H, cfg.BLK], BF16, tag="hr")
                            nc.scalar.activation(
                                out=hr_t[:], in_=h_t[:],
                                func=mybir.ActivationFunctionType.Relu,
                                bias=bc_s[:])
                            if layer == 1:
                                if b % 2 == 0:
                                    pair["p2"] = pp.tile([128, H], F32, name="p2",
                                                         tag="pair")
                                p2 = pair["p2"]
                                nc.tensor.matmul(
                                    out=p2[half:half + H, :], lhsT=hr_t[:],
                                    rhs=w2_s[:], start=True, stop=True,
                                    tile_position=(0, half))
                                if b % 2 == 1:
                                    k, tk = t // TPC, t % TPC
                                    if tk == 0:
                                        pair["stg"] = stg2pool.tile(
                                            [128, TPC, H], BF16, name="stg2",
                                            tag="stg2")
                                    nc.vector.tensor_scalar_mul(
                                        out=pair["stg"][:, tk, :], in0=p2[:],
                                        scalar1=dinvn_s[:, t:t + 1])
                                    nc.vector.tensor_scalar_mul(
                                        out=self2_s[:, t * H:(t + 1) * H],
                                        in0=p2[:],
                                        scalar1=dinvn_s[:, t:t + 1])
                                    if tk == TPC - 1:
                                        nc.scalar.dma_start(
                                            out=stage_out_ap(hs2_c[k]),
                                            in_=pair["stg"][:])
                            else:
                                if b % 2 == 0:
                                    pair["pl"] = pp.tile([128, cfg.N_CLASSES],
                                                         F32, name="pl", tag="pl")
                                pl = pair["pl"]
                                nc.tensor.matmul(
                                    out=pl[half:half + H, :], lhsT=hr_t[:],
                                    rhs=wl_s[:], start=True, stop=True,
                                    tile_position=(0, half))
                                if b % 2 == 1:
                                    nCL = cfg.N_CLASSES
                                    nc.vector.tensor_tensor(
                                        out=stageL_s[:, t * nCL:(t + 1) * nCL],
                                        in0=pl[:], in1=blrep_s[:],
                                        op=mybir.AluOpType.add)

                    if layer == 1:
                        for k in range(NCHUNK):
                            if not fired[k]:
                                fire_tab2(k)

                conv_layer(1)
                conv_layer(2)

            nc.sync.dma_start(out=logits_d[:, :], in_=stageL_s[:])

    nc.compile()
    return nc


_PROGRAM_CACHE = {}


def get_program(cfg):
    key = id(cfg)
    if key not in _PROGRAM_CACHE:
        _PROGRAM_CACHE[key] = build_program(cfg)
    return _PROGRAM_CACHE[key]


def run(cfg, inputs, trace=False):
    in_maps, node_of_slot = preprocess(cfg, **inputs)
    nc = get_program(cfg)
    res = bass_utils.run_bass_kernel_spmd(
        nc, in_maps, core_ids=list(range(cfg.NC)), trace=trace)
    out = assemble_output(cfg, res.results, node_of_slot)
    return out, res


def kernel(**inputs) -> np.ndarray:
    out, _ = run(CFG_FULL, inputs)
    return out

